# revision 2
# baseline (speedup 1.0000x reference)
"""AttnBlock2D (B=4, C=512, H=W=64) on 8 Trainium2 NeuronCores.

Data-parallel over batch x sequence-parallel over output tokens (core c
handles image c//2, output-token half c%2), with all heavy GEMMs in fp8e4m3
DoubleRow and *energy-ordered split precision* via host-side SVD rotations:

  scores:  s[i,j] = x_j^T (Wq^T Wk) x_i  (i-only terms cancel in softmax)
           Wq^T Wk ... M^T = U S V^T (SVD, host).  Send x~ = U^T x (iid
           N(0,1), same stats as x).  Then s = sum_k x~_jk h_ik with
           h = (S V^T U) x~_i, so component k carries energy S_k^2.  The
           top-256 components get the exact-ish 3-product hi/lo fp8
           expansion; the bottom-256 (1.5% of energy) get 1 product.
  apply:   u[c,i] = sum_j w_ij v'_cj, v' = (Wo Wv) x = U' S' V'^T x.
           b' = (S' V'^T U) x~ (sigma-folded, channel-ordered), apply
           contracts b' against e0 = fp8(exp(logits)) -- top-256 channels
           2 products, bottom-256 1 product -- then y = U' (b-avg)
           back-rotation GEMM (top 3p / bottom 1p).  The e_lo correction is
           dropped entirely: numerator and denominator both use e0, which is
           exact softmax of logits perturbed by e's fp8 rounding (~1.2e-2).
  denom:   b' channel 511 is replaced by the constant BETA (memset), so the
           apply GEMM's bottom chunk also produces s_i = BETA sum_j e0[j,i]
           for free; the old ones-reduce matmuls are gone.

Per-core PE work drops from ~483k to ~300k cycles vs the 3-product
baseline; sim rel-err ~1.4e-2 (budget 2e-2).

Layout/scheduling (from the baseline): scores-transposed formulation
(softmax axis j on partitions, zero transposes), hi|lo packed DMAs,
need-ordered sync-queue DMA stream, phase-A/b-proj/pt riding ib0, p-state
warm-up matmul, psum rings psA(3)+po(4)+py(1) = 8 banks.
"""

import numpy as np
import ml_dtypes

import concourse.bass as bass
import concourse.tile as tile
import concourse.mybir as mybir
from concourse import bacc
from concourse.bass_utils import run_bass_kernel_spmd

B = 4
C = 512            # C_IN == C_HID
HW = 64 * 64       # tokens per image
NCORES = 8
I = HW * B // NCORES   # 2048 output tokens per core

CK = 128           # partition chunk
NB = 512           # free-dim block
NCH = C // CK      # 4
NJB = HW // CK     # 32
NIB = I // NB      # 4
XCH = HW // 4      # xj DMA chunk (8 j-chunks)

F32 = mybir.dt.float32
F32R = mybir.dt.float32r
BF16 = mybir.dt.bfloat16
F8 = mybir.dt.float8e4
NP8 = ml_dtypes.float8_e4m3
AF = mybir.ActivationFunctionType
DR = mybir.MatmulPerfMode.DoubleRow
OP = mybir.AluOpType

SCALE = 1.0 / float(np.sqrt(float(C)))
BETA = 16.0        # weight pre-scale: keeps w_lo out of fp8 subnormals
ALPHA = 1024.0     # t-vector pre-scale
SHIFT = -2.0       # global logit shift (cancels in softmax; bounds e)
GAMMA = 16.0       # u^ pre-scale: keeps u^ out of fp8 subnormals

SKEW0 = 4          # ib0 apply skew (jcq)
SKEW = 5           # ib>0 apply skew: covers the u^ normalize DVE/ACT chain
ROT_AT = 4         # jcq at which the previous ib's rotation matmuls drop in


def build_bass():
    nc = bacc.Bacc(
        "TRN2", target_bir_lowering=False, debug=False, enable_asserts=False
    )

    # hi|lo packed inputs: one DMA per block
    xj2 = nc.dram_tensor("xj2", [2, C, HW], F8, kind="ExternalInput").ap()
    xi2 = nc.dram_tensor("xi2", [2, C, I], F8, kind="ExternalInput").ap()
    wm2 = nc.dram_tensor("wm2", [2, C, C], F8, kind="ExternalInput").ap()
    wv2 = nc.dram_tensor("wv2", [2, C, C], F8, kind="ExternalInput").ap()
    rt2 = nc.dram_tensor("rt2", [2, C, C], F8, kind="ExternalInput").ap()
    uph = nc.dram_tensor("uph", [CK, NCH, 2], F8, kind="ExternalInput").ap()
    bop = nc.dram_tensor("bop", [CK, NCH], F32, kind="ExternalInput").ap()
    out = nc.dram_tensor("out", [C, I], BF16, kind="ExternalOutput").ap()

    # DRAM views with the channel dim split for 128-partition DMA
    xj4 = xj2.rearrange("t (a p) n -> p t a n", p=CK)  # [128, 2, 4, 4096]
    xi4 = xi2.rearrange("t (a p) n -> p t a n", p=CK)  # [128, 2, 4, 2048]
    wm4 = wm2.rearrange("t (a p) n -> p t a n", p=CK)  # [128, 2, 4, 512]
    wv4 = wv2.rearrange("t (a p) n -> p t a n", p=CK)
    rt4 = rt2.rearrange("t (a p) n -> p t a n", p=CK)
    out3 = out.rearrange("(a p) n -> p a n", p=CK)     # [128, 4, 2048]

    with tile.TileContext(nc) as tc:
        with tc.tile_pool(name="persist", bufs=1) as persist, \
             tc.tile_pool(name="wp", bufs=1) as wp, \
             tc.tile_pool(name="xp", bufs=4) as xp, \
             tc.tile_pool(name="etp", bufs=8) as etp, \
             tc.tile_pool(name="ftp", bufs=4) as ftp, \
             tc.tile_pool(name="ubp", bufs=2) as ubp, \
             tc.tile_pool(name="rp", bufs=2) as rp, \
             tc.tile_pool(name="psA", bufs=4, space="PSUM") as psA, \
             tc.tile_pool(name="psO", bufs=1, space="PSUM") as psO, \
             tc.tile_pool(name="xj0p", bufs=1) as xj0p, \
             tc.tile_pool(name="xj1p", bufs=1) as xj1p, \
             tc.tile_pool(name="xj2p", bufs=1) as xj2p, \
             tc.tile_pool(name="xj3p", bufs=1) as xj3p:

            # ---- persistent SBUF state ----
            # each xj chunk gets its own single-tile pool so readers only
            # wait on the one chunk DMA that wrote their data (write
            # tracking is pool-granular)
            xjc = [p.tile([CK, 2, NCH, XCH], F8, name=f"xjc{c}")
                   for c, p in enumerate((xj0p, xj1p, xj2p, xj3p))]
            h_hi = persist.tile([CK, NCH, I], F8, name="h_hi")
            h_lo = persist.tile([CK, 2, I], F8, name="h_lo")   # top half only
            vT_hi = persist.tile([CK, NJB, C], F8, name="vT_hi")
            vT_lo = persist.tile([CK, NJB, CK], F8, name="vT_lo")
            tt = persist.tile([CK, NJB], F32, name="tt")
            bop_t = persist.tile([CK, NCH], F32, name="bop_t")
            # up_t lives in the weights pool: reading a tile waits on all
            # earlier writes to its pool, and persist receives the phase-A
            # h evacuations (which would stall the pt burst)
            up_t = wp.tile([CK, NCH, 2], F8, name="up_t")
            wm = wp.tile([CK, 2, NCH, C], F8, name="wm")
            wv = wp.tile([CK, 2, NCH, C], F8, name="wv")
            rt = wp.tile([CK, 2, NCH, C], F8, name="rt")

            # b' channel 256 == BETA/GAMMA: the apply GEMM's cc=2 chunk
            # then emits (BETA/GAMMA) sum_j e0[j,i] on po2 partition 0,
            # whose plain reciprocal is exactly the GAMMA/(BETA s) scale
            # the u^ normalize needs -- no separate rescale op
            nc.vector.memset(vT_hi[:, :, C // 2], BETA / GAMMA)
            # p-state warm-up: a tiny dummy matmul right after the memsets
            # starts the tensor engine's 3us ramp clock at ~0.2us, so the
            # first real matmuls (after the critical DMAs land) already run
            # at full rate
            warm = persist.tile([CK, 2, 32], F8, name="warm")
            nc.vector.memset(warm, BETA)
            dum = psA.tile([32, 32], F32, name="dum", tag="psA",
                           space="PSUM")
            nc.tensor.matmul(dum, lhsT=warm, rhs=warm,
                             start=True, stop=True, perf_mode=DR)

            # DMAs serialize globally in arrival order, so issue everything
            # need-ordered on the sync queue: phase-A inputs first, then the
            # xj chunks and wv interleaved in consumption order
            nc.sync.dma_start(out=wm[:, 0], in_=wm4[:, 0])

            # convenience pair views into the packed xj chunk tiles
            JPC = XCH // CK  # j-chunks per xj chunk tile

            def xjh_p(ccp, jc):
                return xjc[jc // JPC][:, 0, ccp:ccp + 2,
                                      (jc % JPC) * CK:(jc % JPC + 1) * CK]

            def xjl_p(ccp, jc):
                return xjc[jc // JPC][:, 1, ccp:ccp + 2,
                                      (jc % JPC) * CK:(jc % JPC + 1) * CK]

            # DMA stream (sync, need-ordered): wm0, xt0h, xt0l, wm1, xt1,
            # xjc0, xt2, xt3, wv, xjc1..3, rt.  Phase-A blocks themselves
            # run interleaved with ib-0's first four cycles (below).
            xts = []
            for ib in range(NIB):
                xt = xp.tile([CK, 2, NCH, NB], F8, name="xt", tag="xt")
                xts.append(xt)
            def xjc_dma(ch, half):
                lo = ch * XCH + half * (XCH // 2)
                nc.sync.dma_start(
                    out=xjc[ch][:, :, :, half * (XCH // 2):
                                (half + 1) * (XCH // 2)],
                    in_=xj4[:, :, :, lo:lo + XCH // 2])

            # need-ordered stream: phase-A(0) inputs, then xjc0 + wv so the
            # first scores/vt/apply pipeline saturates PE by ~10us; the
            # remaining xt blocks (phase-A 1-3, deferred to jcq 10-14) and
            # xj chunks follow in consumption order
            nc.sync.dma_start(out=xts[0][:, 0], in_=xi4[:, 0, :, 0:NB])
            nc.sync.dma_start(out=up_t, in_=uph)
            nc.sync.dma_start(out=xts[0][:, 1], in_=xi4[:, 1, :, 0:NB])
            nc.sync.dma_start(out=wm[:, 1], in_=wm4[:, 1])
            xjc_dma(0, 0)
            nc.sync.dma_start(out=wv[:, 0], in_=wv4[:, 0])
            xjc_dma(0, 1)
            nc.sync.dma_start(out=wv[:, 1], in_=wv4[:, 1])
            xjc_dma(1, 0)
            xjc_dma(1, 1)
            nc.gpsimd.dma_start(out=bop_t, in_=bop)
            nc.sync.dma_start(out=xts[1], in_=xi4[:, :, :, 1 * NB:2 * NB])
            nc.sync.dma_start(out=xts[2], in_=xi4[:, :, :, 2 * NB:3 * NB])
            nc.sync.dma_start(out=xts[3], in_=xi4[:, :, :, 3 * NB:4 * NB])
            for ch in range(2, 4):
                xjc_dma(ch, 0)
                xjc_dma(ch, 1)
            nc.sync.dma_start(out=rt, in_=rt4)

            def phase_a_block(ab):
                # h = (S V^T U) x~ for one 512-token block of own i.
                # Output rows (= h components) are energy-ordered: top co
                # chunks get the 3-product expansion, bottom 2 products.
                xt = xts[ab]
                for co in range(NCH):
                    # phase-A psums ride the psA ring so the blocks can run
                    # at any point of ib0; casts ACT, subs DVE
                    ph = psA.tile([CK, NB], F32, name=f"ph{co}",
                                  tag="psA", space="PSUM")
                    prods = (((0, 0), (0, 1), (1, 0)) if co < 2
                             else ((0, 0),))
                    first = True
                    for n, (wa, xa) in enumerate(prods):
                        for ccp in (0, 2):
                            nc.tensor.matmul(
                                ph,
                                lhsT=wm[:, wa, ccp:ccp + 2,
                                        co * CK:(co + 1) * CK],
                                rhs=xt[:, xa, ccp:ccp + 2, :],
                                start=first,
                                stop=(n == len(prods) - 1 and ccp == 2),
                                perf_mode=DR,
                            )
                            first = False
                    hh = h_hi[:, co, ab * NB:(ab + 1) * NB]
                    nc.scalar.activation(hh, ph, AF.Copy)
                    if co < 2:
                        hl = h_lo[:, co, ab * NB:(ab + 1) * NB]
                        nc.vector.tensor_sub(hl, ph, hh)

            def pt_burst(jc0, n=4):
                # t[j] = alpha.SCALE.(U^T Wq^T bk).x~_j  (hi-only product);
                # n j-chunks share one psum tile = one psA ring slot.  All
                # matmuls first, then ONE strided DVE op converts the
                # burst -- no PE<->DVE ping-pong on the critical path.
                pt = psA.tile([CK, n, 2], F32, name="pt", tag="psA",
                              space="PSUM")
                for k in range(n):
                    jc = jc0 + k
                    for ccp in (0, 2):
                        nc.tensor.matmul(
                            pt[:, k, :], lhsT=xjh_p(ccp, jc),
                            rhs=up_t[:, ccp:ccp + 2, :],
                            start=(ccp == 0), stop=(ccp == 2), perf_mode=DR,
                        )
                nc.vector.tensor_scalar(
                    tt[:, jc0:jc0 + n], pt[:, :, 0],
                    1.0 / ALPHA, SHIFT, OP.mult, OP.add,
                )

            def vt_gemm(jc):
                # b'[j, k] = (S' V'^T U) x~: top-256 output cols 3-product,
                # bottom-256 1-product; evac ACT hi (cols 0:511) / DVE lo
                # (cols 0:256 -- only the top needs a correction term)
                pv = psA.tile([CK, C], F32, name="pv", tag="psA",
                              space="PSUM")
                # one accumulation group per column range (they may not
                # interleave within a psum bank): top 3-product group first,
                # then the bottom 1-product group
                first = True
                for (xa, wa) in ((xjh_p, 0), (xjh_p, 1), (xjl_p, 0)):
                    for ccp in (0, 2):
                        nc.tensor.matmul(
                            pv[:, 0:C // 2], lhsT=xa(ccp, jc),
                            rhs=wv[:, wa, ccp:ccp + 2, 0:C // 2],
                            start=first,
                            stop=(xa is xjl_p and ccp == 2),
                            perf_mode=DR)
                        first = False
                nc.tensor.matmul(
                    pv[:, C // 2:C], lhsT=xjh_p(0, jc),
                    rhs=wv[:, 0, 0:2, C // 2:C],
                    start=True, stop=False, perf_mode=DR)
                nc.tensor.matmul(
                    pv[:, C // 2:C], lhsT=xjh_p(2, jc),
                    rhs=wv[:, 0, 2:4, C // 2:C],
                    start=False, stop=True, perf_mode=DR)
                # hi evac split ACT/DVE (GPSIMD cannot read PSUM): keeps
                # either engine under PE's ib0 rate.  Column 256 (the
                # memset ones channel) is skipped by both ranges.
                nc.scalar.activation(vT_hi[:, jc, 0:C // 2],
                                     pv[:, 0:C // 2], AF.Copy)
                nc.vector.tensor_scalar_add(vT_hi[:, jc, C // 2 + 1:C],
                                            pv[:, C // 2 + 1:C], 0.0)
                nc.vector.tensor_sub(vT_lo[:, jc, :], pv[:, 0:CK],
                                     vT_hi[:, jc, 0:CK])

            def emit_rotation(uh, ul, ib):
                # back-rotation y = (BETA U')^T u^ in 4 psum banks with
                # open accumulation groups: all top-component matmuls
                # (which only need uh/ul chunks 0,1) run first, so the
                # bottom matmuls wait on the late uh chunks 2,3 with the
                # PE already fed; evacs split DVE/ACT to shorten the tail
                pys = []
                for co in range(NCH):
                    py = psA.tile([CK, NB], F32, name="py", tag="psA",
                                  space="PSUM")
                    pys.append(py)
                    nc.tensor.matmul(
                        py, lhsT=rt[:, 0, 0:2, co * CK:(co + 1) * CK],
                        rhs=uh[:, 0:2, :], start=True, stop=False,
                        perf_mode=DR)
                    nc.tensor.matmul(
                        py, lhsT=rt[:, 0, 0:2, co * CK:(co + 1) * CK],
                        rhs=ul[:, 0:2, :], start=False, stop=False,
                        perf_mode=DR)
                    nc.tensor.matmul(
                        py, lhsT=rt[:, 1, 0:2, co * CK:(co + 1) * CK],
                        rhs=uh[:, 0:2, :], start=False, stop=False,
                        perf_mode=DR)
                for co in range(NCH):
                    py = pys[co]
                    nc.tensor.matmul(
                        py, lhsT=rt[:, 0, 2:4, co * CK:(co + 1) * CK],
                        rhs=uh[:, 2:4, :], start=False, stop=True,
                        perf_mode=DR)
                    ftb = ftp.tile([CK, NB], BF16, name="ftb", tag="ftb")
                    if co % 2 == 0:
                        nc.vector.tensor_scalar(ftb, py,
                                                1.0 / (BETA * GAMMA),
                                                bop_t[:, co:co + 1],
                                                OP.mult, OP.add)
                    else:
                        nc.scalar.activation(ftb, py, AF.Identity,
                                             scale=1.0 / (BETA * GAMMA),
                                             bias=bop_t[:, co:co + 1])
                    nc.sync.dma_start(
                        out=out3[:, co, ib * NB:(ib + 1) * NB], in_=ftb)

            # ---- phase C: scores, exp, apply per 512-token i-block ----
            pending_rot = None
            for ib in range(NIB):
                po = [
                    psO.tile([CK, NB], F32, name=f"po{cc}", tag=f"po{cc}",
                             space="PSUM")
                    for cc in range(NCH)
                ]

                def apply_cc(jcq, et, cc):
                    # b'-lo correction only on the top-128 channels (cc=0):
                    # channels 128..255 carry ~13% of the energy, so their
                    # b-quantization noise is already small
                    jc0 = 2 * jcq
                    first = jcq == 0
                    last = jcq == NJB // 2 - 1
                    vh = vT_hi[:, jc0:jc0 + 2, cc * CK:(cc + 1) * CK]
                    nc.tensor.matmul(
                        po[cc], lhsT=vh, rhs=et[:, :, :],
                        start=first, stop=(last and cc != 0),
                        perf_mode=DR,
                    )
                    if cc == 0:
                        vl = vT_lo[:, jc0:jc0 + 2, cc * CK:(cc + 1) * CK]
                        nc.tensor.matmul(
                            po[cc], lhsT=vl, rhs=et[:, :, :],
                            start=False, stop=last, perf_mode=DR,
                        )

                def apply_jcq(jcq, et):
                    # u~[k, i] += b'[j, k] e0[j, i]: top chunks 2-product,
                    # bottom chunks 1-product (incl. the ones channel)
                    for cc in range(NCH):
                        apply_cc(jcq, et, cc)

                pending = []
                skew = SKEW0 if ib == 0 else SKEW
                for jcq in range(NJB // 2):
                    if ib == 0 and jcq == 0:
                        phase_a_block(0)
                    if ib == 0 and jcq in (10, 12, 14):
                        # phase-A blocks 1-3 ride ib-0's later cycles,
                        # well after their xt DMAs have landed
                        phase_a_block((jcq - 8) // 2)
                    if ib == 0 and jcq % 2 == 0:
                        pt_burst(2 * jcq, n=4)
                    if jcq == ROT_AT and pending_rot is not None:
                        # the previous ib's back-rotation matmuls drop in
                        # here, after a jcq of scores: by now the DVE/ACT
                        # u^ normalize+split chain has landed, so PE never
                        # waits on it
                        emit_rotation(*pending_rot)
                        pending_rot = None
                    et = etp.tile([CK, 2, NB], F8, name="et", tag="et")
                    for q in (0, 1):
                        jc = 2 * jcq + q
                        if ib == 0:
                            # b' before each scores half: interleaves the
                            # pv/ps psum ring and gives the phase-A evac
                            # chain time to land before scores reads h
                            vt_gemm(jc)
                        ps_ = psA.tile([CK, NB], F32, name="ps", tag="psA",
                                       space="PSUM")
                        # top components 3-product, bottom 1-product
                        nc.tensor.matmul(
                            ps_, lhsT=xjh_p(0, jc),
                            rhs=h_hi[:, 0:2, ib * NB:(ib + 1) * NB],
                            start=True, stop=False, perf_mode=DR)
                        nc.tensor.matmul(
                            ps_, lhsT=xjh_p(0, jc),
                            rhs=h_lo[:, 0:2, ib * NB:(ib + 1) * NB],
                            start=False, stop=False, perf_mode=DR)
                        # the x~-lo correction covers 3/4 of the i-block
                        # (F=384): cheaper, and the remaining quarter's
                        # noise is well inside the error budget
                        nc.tensor.matmul(
                            ps_[:, 0:384], lhsT=xjl_p(0, jc),
                            rhs=h_hi[:, 0:2, ib * NB:ib * NB + 384],
                            start=False, stop=False, perf_mode=DR)
                        nc.tensor.matmul(
                            ps_, lhsT=xjh_p(2, jc),
                            rhs=h_hi[:, 2:4, ib * NB:(ib + 1) * NB],
                            start=False, stop=True, perf_mode=DR)
                        # e0 = fp8(exp(logits)) straight from ScalarE
                        nc.scalar.activation(
                            et[:, q, :], ps_, AF.Exp,
                            scale=SCALE / BETA, bias=tt[:, jc:jc + 1])
                    pending.append((jcq, et))
                    # issue skew: PE runs scores(jcq+1..) while the ACT
                    # exp pipe finishes e0(jcq)
                    if len(pending) > skew:
                        apply_jcq(*pending.pop(0))
                # flush cc-major with the denominator chunk first: po2
                # (and then each po[cc]) completes early, hiding the
                # reciprocal/broadcast/normalize chain behind the
                # remaining apply matmuls
                for cc in (2, 0, 1, 3):
                    for p in pending:
                        apply_cc(*p, cc)
                pending = []

                # normalise: r[i] = GAMMA / (BETA sum_j e0[j,i]) (po3
                # partition 127 holds the ones-channel sum), u^ = u~ . r,
                # split hi/lo.  The rotation matmuls are deferred into the
                # next ib's loop so PE chews scores while this DVE/ACT
                # chain lands (last ib: emitted right here).
                r1 = rp.tile([1, NB], F32, name="r1", tag="r1")
                nc.vector.reciprocal(r1, po[2][0:1, :])
                rb = rp.tile([CK, NB], F32, name="rb", tag="rb")
                nc.gpsimd.partition_broadcast(rb, r1)
                uh = ubp.tile([CK, NCH, NB], F8, name="uh", tag="uh")
                ul = ubp.tile([CK, 2, NB], F8, name="ul", tag="ul")
                last_ib = ib == NIB - 1
                for cc in range(NCH):
                    ft = ftp.tile([CK, NB], F32R, name="ft", tag="ft")
                    nc.vector.tensor_mul(ft, po[cc], rb)
                    if last_ib:
                        # ACT is idle at the tail; its lower latency
                        # shortens the final normalize->rotate chain
                        nc.scalar.activation(uh[:, cc, :], ft, AF.Copy)
                    else:
                        nc.gpsimd.tensor_copy(uh[:, cc, :], ft)
                    if cc < 2:
                        nc.vector.tensor_sub(ul[:, cc, :], ft, uh[:, cc, :])
                if ib < NIB - 1:
                    pending_rot = (uh, ul, ib)
                else:
                    emit_rotation(uh, ul, ib)

    nc.compile()
    return nc


_NC = None


def _get_nc():
    global _NC
    if _NC is None:
        _NC = build_bass()
    return _NC


def _split8(a):
    hi = np.asarray(a, NP8)
    lo = np.asarray(a - hi.astype(np.float32), NP8)
    return np.ascontiguousarray(np.stack([hi, lo]))


def _make_in_maps(inp, Wk, bk, Wq, bq, Wv, bv, Wo, bo):
    # host-side SVD rotations (f64) + folded weights; the scores bilinear
    # form is x_j^T M^T x_i, so rotate with the SVD of M^T
    M64 = (np.asarray(Wk, np.float64).T @ np.asarray(Wq, np.float64))
    U, sv, Vt = np.linalg.svd(M64.T)
    A64 = (np.asarray(Wo, np.float64) @ np.asarray(Wv, np.float64))
    Up, sp, Vpt = np.linalg.svd(A64)

    # h = (S V^T U) x~; kernel lhsT layout wants [c_in, k_out]
    Wh = (BETA * (np.diag(sv) @ Vt @ U)).astype(np.float32)
    wm2_ = _split8(np.ascontiguousarray(Wh.T))
    # b' = (S' V'^T U) x~ -> [c_in, k_out].  The ones channel must sit at
    # partition 0 of an apply psum chunk (engines can't start at partition
    # 127), so it lives at column 256: components 256..510 shift up one
    # column and the weakest component (sigma'_511) is dropped.
    Wb0 = (BETA * (U.T @ Vpt.T @ np.diag(sp))).astype(np.float32)
    Wb = np.zeros_like(Wb0)
    Wb[:, :256] = Wb0[:, :256]
    Wb[:, 257:] = Wb0[:, 256:C - 1]
    wv2_ = _split8(np.ascontiguousarray(Wb))
    # y = (BETA U')^T u^ / (BETA GAMMA); lhsT layout [k, c_out], rows
    # permuted to match (row 256 = ones channel = zero contribution)
    Rot0 = (BETA * Up.T).astype(np.float32)
    RotT = np.zeros_like(Rot0)
    RotT[:256] = Rot0[:256]
    RotT[257:] = Rot0[256:C - 1]
    rt2_ = _split8(np.ascontiguousarray(RotT))

    u_eff = (ALPHA * SCALE) * (U.T @ (np.asarray(Wq, np.float64).T
                                      @ np.asarray(bk, np.float64)))
    up2 = np.zeros((CK, NCH, 2), np.float32)
    up2[:, :, 0] = u_eff.astype(np.float32).reshape(NCH, CK).T
    uph_ = np.ascontiguousarray(up2.astype(NP8))

    bo_eff = (np.asarray(Wo, np.float32) @ np.asarray(bv, np.float32)
              + np.asarray(bo, np.float32))
    bop_ = np.ascontiguousarray(bo_eff.reshape(NCH, CK).T)

    x_all = np.asarray(inp, dtype=np.float32).reshape(B, C, HW)
    xsplit = [
        _split8((U.T @ x_all[b].astype(np.float64)).astype(np.float32))
        for b in range(B)
    ]

    in_maps = []
    for c in range(NCORES):
        b, h = divmod(c, NCORES // B)
        x2 = xsplit[b]
        in_maps.append({
            "xj2": x2,
            "xi2": np.ascontiguousarray(x2[:, :, h * I:(h + 1) * I]),
            "wm2": wm2_, "wv2": wv2_, "rt2": rt2_,
            "uph": uph_, "bop": bop_,
        })
    return in_maps


def run(trace=False, tmpdir=None, **inputs):
    nc = _get_nc()
    in_maps = _make_in_maps(**inputs)
    res = run_bass_kernel_spmd(
        nc, in_maps, core_ids=list(range(NCORES)), trace=trace, tmpdir=tmpdir
    )
    full = np.empty((B, C, HW), dtype=np.float32)
    for c in range(NCORES):
        b, h = divmod(c, NCORES // B)
        full[b][:, h * I:(h + 1) * I] = (
            res.results[c]["out"].astype(np.float32))
    return full.reshape(B, C, 64, 64), res


def kernel(**inputs):
    out, _ = run(trace=False, **inputs)
    return out


# revision 4
# speedup vs baseline: 1.0344x; 1.0344x over previous
"""AttnBlock2D (B=4, C=512, H=W=64) on 8 Trainium2 NeuronCores.

Data-parallel over batch x sequence-parallel over output tokens (core c
handles image c//2, output-token half c%2), with all heavy GEMMs in fp8e4m3
DoubleRow and *energy-ordered split precision* via host-side SVD rotations:

  scores:  s[i,j] = x_j^T (Wq^T Wk) x_i  (i-only terms cancel in softmax)
           Wq^T Wk ... M^T = U S V^T (SVD, host).  Send x~ = U^T x (iid
           N(0,1), same stats as x).  Then s = sum_k x~_jk h_ik with
           h = (S V^T U) x~_i, so component k carries energy S_k^2.  The
           top-256 components get the exact-ish 3-product hi/lo fp8
           expansion; the bottom-256 (1.5% of energy) get 1 product.
  apply:   u[c,i] = sum_j w_ij v'_cj, v' = (Wo Wv) x = U' S' V'^T x.
           b' = (S' V'^T U) x~ (sigma-folded, channel-ordered), apply
           contracts b' against e0 = fp8(exp(logits)) -- 2 products on the
           top-128 channels, 1 product elsewhere -- then y = U' (b-avg)
           back-rotation GEMM (top 3p / bottom 1p).  The e_lo correction is
           dropped entirely: numerator and denominator both use e0, which is
           exact softmax of logits perturbed by e's fp8 rounding (~1.2e-2).
  denom:   b' channel 256 is the constant BETA/GAMMA (memset; partition 0
           of the po2 chunk, where engines may read), so the apply GEMM
           also produces the softmax denominator for free and its plain
           DVE reciprocal is exactly the u^ normalize scale; the old
           ones-reduce matmuls are gone.

Per-core PE work drops from ~483k to ~271k cycles vs the 3-product
baseline; sim/HW rel-err 1.63e-2 (budget 2e-2; sim and HW agree to
+-3e-6 on this fixed-seed problem).

Layout/scheduling (evolved from the baseline): scores-transposed
formulation (softmax axis j on partitions, zero transposes), hi|lo packed
DMAs with half-chunk xj transfers need-ordered on the sync queue, b-proj/
pt riding ib0 with phase-A blocks at jcq 0/10/12/14 (matching their DMA
arrivals), p-state warm-up matmul, psum layout psA-ring(4)+po(4) = 8
banks with phase-A/back-rotation psums on the psA ring, per-ib
back-rotation matmuls deferred into the next ib (PE never waits on the
normalize chain), cc-major flush with the denominator chunk first, evac
work balanced ACT/DVE/Pool around the ScalarE exp stream.
"""

import numpy as np
import ml_dtypes

import concourse.bass as bass
import concourse.tile as tile
import concourse.mybir as mybir
from concourse import bacc
from concourse.bass_utils import run_bass_kernel_spmd

B = 4
C = 512            # C_IN == C_HID
HW = 64 * 64       # tokens per image
NCORES = 8
I = HW * B // NCORES   # 2048 output tokens per core

CK = 128           # partition chunk
NB = 512           # free-dim block
NCH = C // CK      # 4
NJB = HW // CK     # 32
NIB = I // NB      # 4
XCH = HW // 4      # xj DMA chunk (8 j-chunks)

F32 = mybir.dt.float32
F32R = mybir.dt.float32r
BF16 = mybir.dt.bfloat16
F8 = mybir.dt.float8e4
NP8 = ml_dtypes.float8_e4m3
AF = mybir.ActivationFunctionType
DR = mybir.MatmulPerfMode.DoubleRow
OP = mybir.AluOpType

SCALE = 1.0 / float(np.sqrt(float(C)))
BETA = 16.0        # weight pre-scale: keeps w_lo out of fp8 subnormals
ALPHA = 1024.0     # t-vector pre-scale
SHIFT = -2.0       # global logit shift (cancels in softmax; bounds e)
GAMMA = 16.0       # u^ pre-scale: keeps u^ out of fp8 subnormals

SKEW0 = 4          # ib0 apply skew (jcq)
SKEW = 5           # ib>0 apply skew: covers the u^ normalize DVE/ACT chain
ROT_AT = 4         # jcq at which the previous ib's rotation matmuls drop in


def build_bass():
    nc = bacc.Bacc(
        "TRN2", target_bir_lowering=False, debug=False, enable_asserts=False
    )

    # hi|lo packed inputs: one DMA per block
    xj2 = nc.dram_tensor("xj2", [2, C, HW], F8, kind="ExternalInput").ap()
    xi2 = nc.dram_tensor("xi2", [2, C, I], F8, kind="ExternalInput").ap()
    wm2 = nc.dram_tensor("wm2", [2, C, C], F8, kind="ExternalInput").ap()
    wv2 = nc.dram_tensor("wv2", [2, C, C], F8, kind="ExternalInput").ap()
    rt2 = nc.dram_tensor("rt2", [2, C, C], F8, kind="ExternalInput").ap()
    uph = nc.dram_tensor("uph", [CK, NCH, 2], F8, kind="ExternalInput").ap()
    bop = nc.dram_tensor("bop", [CK, NCH], F32, kind="ExternalInput").ap()
    out = nc.dram_tensor("out", [C, I], BF16, kind="ExternalOutput").ap()

    # DRAM views with the channel dim split for 128-partition DMA
    xj4 = xj2.rearrange("t (a p) n -> p t a n", p=CK)  # [128, 2, 4, 4096]
    xi4 = xi2.rearrange("t (a p) n -> p t a n", p=CK)  # [128, 2, 4, 2048]
    wm4 = wm2.rearrange("t (a p) n -> p t a n", p=CK)  # [128, 2, 4, 512]
    wv4 = wv2.rearrange("t (a p) n -> p t a n", p=CK)
    rt4 = rt2.rearrange("t (a p) n -> p t a n", p=CK)
    out3 = out.rearrange("(a p) n -> p a n", p=CK)     # [128, 4, 2048]

    with tile.TileContext(nc) as tc:
        with tc.tile_pool(name="persist", bufs=1) as persist, \
             tc.tile_pool(name="wp", bufs=1) as wp, \
             tc.tile_pool(name="xp", bufs=4) as xp, \
             tc.tile_pool(name="etp", bufs=8) as etp, \
             tc.tile_pool(name="ftp", bufs=4) as ftp, \
             tc.tile_pool(name="ubp", bufs=2) as ubp, \
             tc.tile_pool(name="rp", bufs=2) as rp, \
             tc.tile_pool(name="psA", bufs=4, space="PSUM") as psA, \
             tc.tile_pool(name="psO", bufs=1, space="PSUM") as psO, \
             tc.tile_pool(name="xj0p", bufs=1) as xj0p, \
             tc.tile_pool(name="xj1p", bufs=1) as xj1p, \
             tc.tile_pool(name="xj2p", bufs=1) as xj2p, \
             tc.tile_pool(name="xj3p", bufs=1) as xj3p:

            # ---- persistent SBUF state ----
            # each xj chunk gets its own single-tile pool so readers only
            # wait on the one chunk DMA that wrote their data (write
            # tracking is pool-granular)
            xjc = [p.tile([CK, 2, NCH, XCH], F8, name=f"xjc{c}")
                   for c, p in enumerate((xj0p, xj1p, xj2p, xj3p))]
            h_hi = persist.tile([CK, NCH, I], F8, name="h_hi")
            h_lo = persist.tile([CK, 2, I], F8, name="h_lo")   # top half only
            vT_hi = persist.tile([CK, NJB, C], F8, name="vT_hi")
            vT_lo = persist.tile([CK, NJB, CK], F8, name="vT_lo")
            tt = persist.tile([CK, NJB], F32, name="tt")
            bop_t = persist.tile([CK, NCH], F32, name="bop_t")
            # up_t lives in the weights pool: reading a tile waits on all
            # earlier writes to its pool, and persist receives the phase-A
            # h evacuations (which would stall the pt burst)
            up_t = wp.tile([CK, NCH, 2], F8, name="up_t")
            wm = wp.tile([CK, 2, NCH, C], F8, name="wm")
            wv = wp.tile([CK, 2, NCH, C], F8, name="wv")
            rt = wp.tile([CK, 2, NCH, C], F8, name="rt")

            # b' channel 256 == BETA/GAMMA: the apply GEMM's cc=2 chunk
            # then emits (BETA/GAMMA) sum_j e0[j,i] on po2 partition 0,
            # whose plain reciprocal is exactly the GAMMA/(BETA s) scale
            # the u^ normalize needs -- no separate rescale op
            nc.vector.memset(vT_hi[:, :, C // 2], BETA / GAMMA)
            # p-state warm-up: a tiny dummy matmul right after the memsets
            # starts the tensor engine's 3us ramp clock at ~0.2us, so the
            # first real matmuls (after the critical DMAs land) already run
            # at full rate
            warm = persist.tile([CK, 2, 32], F8, name="warm")
            nc.vector.memset(warm, BETA)
            dum = psA.tile([32, 32], F32, name="dum", tag="psA",
                           space="PSUM")
            nc.tensor.matmul(dum, lhsT=warm, rhs=warm,
                             start=True, stop=True, perf_mode=DR)

            # DMAs serialize globally in arrival order, so issue everything
            # need-ordered on the sync queue: phase-A inputs first, then the
            # xj chunks and wv interleaved in consumption order
            nc.sync.dma_start(out=wm[:, 0], in_=wm4[:, 0])

            # convenience pair views into the packed xj chunk tiles
            JPC = XCH // CK  # j-chunks per xj chunk tile

            def xjh_p(ccp, jc):
                return xjc[jc // JPC][:, 0, ccp:ccp + 2,
                                      (jc % JPC) * CK:(jc % JPC + 1) * CK]

            def xjl_p(ccp, jc):
                return xjc[jc // JPC][:, 1, ccp:ccp + 2,
                                      (jc % JPC) * CK:(jc % JPC + 1) * CK]

            # DMA stream (sync, need-ordered): wm0, xt0h, xt0l, wm1, xt1,
            # xjc0, xt2, xt3, wv, xjc1..3, rt.  Phase-A blocks themselves
            # run interleaved with ib-0's first four cycles (below).
            xts = []
            for ib in range(NIB):
                xt = xp.tile([CK, 2, NCH, NB], F8, name="xt", tag="xt")
                xts.append(xt)
            def xjc_dma(ch, half):
                lo = ch * XCH + half * (XCH // 2)
                nc.sync.dma_start(
                    out=xjc[ch][:, :, :, half * (XCH // 2):
                                (half + 1) * (XCH // 2)],
                    in_=xj4[:, :, :, lo:lo + XCH // 2])

            # need-ordered stream: phase-A(0) inputs, then xjc0 + wv so the
            # first scores/vt/apply pipeline saturates PE by ~10us; the
            # remaining xt blocks (phase-A 1-3, deferred to jcq 10-14) and
            # xj chunks follow in consumption order
            nc.sync.dma_start(out=xts[0][:, 0], in_=xi4[:, 0, :, 0:NB])
            nc.sync.dma_start(out=up_t, in_=uph)
            nc.sync.dma_start(out=xts[0][:, 1], in_=xi4[:, 1, :, 0:NB])
            nc.sync.dma_start(out=wm[:, 1], in_=wm4[:, 1])
            xjc_dma(0, 0)
            nc.sync.dma_start(out=wv[:, 0], in_=wv4[:, 0])
            xjc_dma(0, 1)
            nc.sync.dma_start(out=wv[:, 1], in_=wv4[:, 1])
            xjc_dma(1, 0)
            xjc_dma(1, 1)
            nc.gpsimd.dma_start(out=bop_t, in_=bop)
            nc.sync.dma_start(out=xts[1], in_=xi4[:, :, :, 1 * NB:2 * NB])
            nc.sync.dma_start(out=xts[2], in_=xi4[:, :, :, 2 * NB:3 * NB])
            nc.sync.dma_start(out=xts[3], in_=xi4[:, :, :, 3 * NB:4 * NB])
            for ch in range(2, 4):
                xjc_dma(ch, 0)
                xjc_dma(ch, 1)
            nc.sync.dma_start(out=rt, in_=rt4)

            def phase_a_block(ab):
                # h = (S V^T U) x~ for one 512-token block of own i.
                # Output rows (= h components) are energy-ordered: top co
                # chunks get the 3-product expansion, bottom 2 products.
                xt = xts[ab]
                for co in range(NCH):
                    # phase-A psums ride the psA ring so the blocks can run
                    # at any point of ib0; casts ACT, subs DVE
                    ph = psA.tile([CK, NB], F32, name=f"ph{co}",
                                  tag="psA", space="PSUM")
                    prods = (((0, 0), (0, 1), (1, 0)) if co < 2
                             else ((0, 0),))
                    first = True
                    for n, (wa, xa) in enumerate(prods):
                        for ccp in (0, 2):
                            nc.tensor.matmul(
                                ph,
                                lhsT=wm[:, wa, ccp:ccp + 2,
                                        co * CK:(co + 1) * CK],
                                rhs=xt[:, xa, ccp:ccp + 2, :],
                                start=first,
                                stop=(n == len(prods) - 1 and ccp == 2),
                                perf_mode=DR,
                            )
                            first = False
                    hh = h_hi[:, co, ab * NB:(ab + 1) * NB]
                    nc.scalar.activation(hh, ph, AF.Copy)
                    if co < 2:
                        hl = h_lo[:, co, ab * NB:(ab + 1) * NB]
                        nc.vector.tensor_sub(hl, ph, hh)

            def pt_burst(jc0, n=4):
                # t[j] = alpha.SCALE.(U^T Wq^T bk).x~_j  (hi-only product);
                # n j-chunks share one psum tile = one psA ring slot.  All
                # matmuls first, then ONE strided DVE op converts the
                # burst -- no PE<->DVE ping-pong on the critical path.
                pt = psA.tile([CK, n, 2], F32, name="pt", tag="psA",
                              space="PSUM")
                for k in range(n):
                    jc = jc0 + k
                    for ccp in (0, 2):
                        nc.tensor.matmul(
                            pt[:, k, :], lhsT=xjh_p(ccp, jc),
                            rhs=up_t[:, ccp:ccp + 2, :],
                            start=(ccp == 0), stop=(ccp == 2), perf_mode=DR,
                        )
                nc.vector.tensor_scalar(
                    tt[:, jc0:jc0 + n], pt[:, :, 0],
                    1.0 / ALPHA, SHIFT, OP.mult, OP.add,
                )

            def vt_gemm(jc):
                # b'[j, k] = (S' V'^T U) x~: top-256 output cols 3-product,
                # bottom-256 1-product; evac ACT hi (cols 0:511) / DVE lo
                # (cols 0:256 -- only the top needs a correction term)
                pv = psA.tile([CK, C], F32, name="pv", tag="psA",
                              space="PSUM")
                # one accumulation group per column range (they may not
                # interleave within a psum bank): top 3-product group first,
                # then the bottom 1-product group
                first = True
                for (xa, wa) in ((xjh_p, 0), (xjh_p, 1), (xjl_p, 0)):
                    for ccp in (0, 2):
                        nc.tensor.matmul(
                            pv[:, 0:C // 2], lhsT=xa(ccp, jc),
                            rhs=wv[:, wa, ccp:ccp + 2, 0:C // 2],
                            start=first,
                            stop=(xa is xjl_p and ccp == 2),
                            perf_mode=DR)
                        first = False
                nc.tensor.matmul(
                    pv[:, C // 2:C], lhsT=xjh_p(0, jc),
                    rhs=wv[:, 0, 0:2, C // 2:C],
                    start=True, stop=False, perf_mode=DR)
                nc.tensor.matmul(
                    pv[:, C // 2:C], lhsT=xjh_p(2, jc),
                    rhs=wv[:, 0, 2:4, C // 2:C],
                    start=False, stop=True, perf_mode=DR)
                # hi evac split ACT/DVE (GPSIMD cannot read PSUM): keeps
                # either engine under PE's ib0 rate.  Column 256 (the
                # memset ones channel) is skipped by both ranges.
                nc.scalar.activation(vT_hi[:, jc, 0:C // 2],
                                     pv[:, 0:C // 2], AF.Copy)
                nc.vector.tensor_scalar_add(vT_hi[:, jc, C // 2 + 1:C],
                                            pv[:, C // 2 + 1:C], 0.0)
                nc.vector.tensor_sub(vT_lo[:, jc, :], pv[:, 0:CK],
                                     vT_hi[:, jc, 0:CK])

            def emit_rotation(uh, ul, ib):
                # back-rotation y = (BETA U')^T u^ in 4 psum banks with
                # open accumulation groups: all top-component matmuls
                # (which only need uh/ul chunks 0,1) run first, so the
                # bottom matmuls wait on the late uh chunks 2,3 with the
                # PE already fed; evacs split DVE/ACT to shorten the tail
                # bottom-component products first: their u^ chunks (2,3)
                # finish earliest in the (2,3,0,1)-ordered flush, so these
                # matmuls feed PE while the top-set normalize still lands
                pys = []
                for co in range(NCH):
                    py = psA.tile([CK, NB], F32, name="py", tag="psA",
                                  space="PSUM")
                    pys.append(py)
                    nc.tensor.matmul(
                        py, lhsT=rt[:, 0, 2:4, co * CK:(co + 1) * CK],
                        rhs=uh[:, 2:4, :], start=True, stop=False,
                        perf_mode=DR)
                for co in range(NCH):
                    py = pys[co]
                    nc.tensor.matmul(
                        py, lhsT=rt[:, 0, 0:2, co * CK:(co + 1) * CK],
                        rhs=uh[:, 0:2, :], start=False, stop=False,
                        perf_mode=DR)
                    nc.tensor.matmul(
                        py, lhsT=rt[:, 0, 0:2, co * CK:(co + 1) * CK],
                        rhs=ul[:, 0:2, :], start=False, stop=False,
                        perf_mode=DR)
                    nc.tensor.matmul(
                        py, lhsT=rt[:, 1, 0:2, co * CK:(co + 1) * CK],
                        rhs=uh[:, 0:2, :], start=False, stop=True,
                        perf_mode=DR)
                    ftb = ftp.tile([CK, NB], BF16, name="ftb", tag="ftb")
                    if co % 2 == 0:
                        nc.vector.tensor_scalar(ftb, py,
                                                1.0 / (BETA * GAMMA),
                                                bop_t[:, co:co + 1],
                                                OP.mult, OP.add)
                    else:
                        nc.scalar.activation(ftb, py, AF.Identity,
                                             scale=1.0 / (BETA * GAMMA),
                                             bias=bop_t[:, co:co + 1])
                    nc.sync.dma_start(
                        out=out3[:, co, ib * NB:(ib + 1) * NB], in_=ftb)

            # ---- phase C: scores, exp, apply per 512-token i-block ----
            pending_rot = None
            for ib in range(NIB):
                po = [
                    psO.tile([CK, NB], F32, name=f"po{cc}", tag=f"po{cc}",
                             space="PSUM")
                    for cc in range(NCH)
                ]

                def apply_cc(jcq, et, cc):
                    # b'-lo correction only on the top-128 channels (cc=0):
                    # channels 128..255 carry ~13% of the energy, so their
                    # b-quantization noise is already small
                    jc0 = 2 * jcq
                    first = jcq == 0
                    last = jcq == NJB // 2 - 1
                    vh = vT_hi[:, jc0:jc0 + 2, cc * CK:(cc + 1) * CK]
                    nc.tensor.matmul(
                        po[cc], lhsT=vh, rhs=et[:, :, :],
                        start=first, stop=(last and cc != 0),
                        perf_mode=DR,
                    )
                    if cc == 0:
                        vl = vT_lo[:, jc0:jc0 + 2, cc * CK:(cc + 1) * CK]
                        nc.tensor.matmul(
                            po[cc], lhsT=vl, rhs=et[:, :, :],
                            start=False, stop=last, perf_mode=DR,
                        )

                def apply_jcq(jcq, et):
                    # u~[k, i] += b'[j, k] e0[j, i]: top chunks 2-product,
                    # bottom chunks 1-product (incl. the ones channel)
                    for cc in range(NCH):
                        apply_cc(jcq, et, cc)

                pending = []
                skew = SKEW0 if ib == 0 else SKEW
                for jcq in range(NJB // 2):
                    if ib == 0 and jcq == 0:
                        phase_a_block(0)
                    if ib == 0 and jcq in (10, 12, 14):
                        # phase-A blocks 1-3 ride ib-0's later cycles,
                        # well after their xt DMAs have landed
                        phase_a_block((jcq - 8) // 2)
                    if ib == 0 and jcq % 2 == 0:
                        pt_burst(2 * jcq, n=4)
                    if jcq == ROT_AT and pending_rot is not None:
                        # the previous ib's back-rotation matmuls drop in
                        # here, after a jcq of scores: by now the DVE/ACT
                        # u^ normalize+split chain has landed, so PE never
                        # waits on it
                        emit_rotation(*pending_rot)
                        pending_rot = None
                    et = etp.tile([CK, 2, NB], F8, name="et", tag="et")
                    for q in (0, 1):
                        jc = 2 * jcq + q
                        if ib == 0:
                            # b' before each scores half: interleaves the
                            # pv/ps psum ring and gives the phase-A evac
                            # chain time to land before scores reads h
                            vt_gemm(jc)
                        ps_ = psA.tile([CK, NB], F32, name="ps", tag="psA",
                                       space="PSUM")
                        # top components 3-product, bottom 1-product
                        nc.tensor.matmul(
                            ps_, lhsT=xjh_p(0, jc),
                            rhs=h_hi[:, 0:2, ib * NB:(ib + 1) * NB],
                            start=True, stop=False, perf_mode=DR)
                        nc.tensor.matmul(
                            ps_, lhsT=xjh_p(0, jc),
                            rhs=h_lo[:, 0:2, ib * NB:(ib + 1) * NB],
                            start=False, stop=False, perf_mode=DR)
                        # the x~-lo correction covers 3/4 of the i-block
                        # (F=384): cheaper, and the remaining quarter's
                        # noise is well inside the error budget
                        nc.tensor.matmul(
                            ps_[:, 0:384], lhsT=xjl_p(0, jc),
                            rhs=h_hi[:, 0:2, ib * NB:ib * NB + 384],
                            start=False, stop=False, perf_mode=DR)
                        nc.tensor.matmul(
                            ps_, lhsT=xjh_p(2, jc),
                            rhs=h_hi[:, 2:4, ib * NB:(ib + 1) * NB],
                            start=False, stop=True, perf_mode=DR)
                        # e0 = fp8(exp(logits)) straight from ScalarE
                        nc.scalar.activation(
                            et[:, q, :], ps_, AF.Exp,
                            scale=SCALE / BETA, bias=tt[:, jc:jc + 1])
                    pending.append((jcq, et))
                    # issue skew: PE runs scores(jcq+1..) while the ACT
                    # exp pipe finishes e0(jcq)
                    if len(pending) > skew:
                        apply_jcq(*pending.pop(0))
                # flush cc-major with the denominator chunk first: po2
                # (and then each po[cc]) completes early, hiding the
                # reciprocal/broadcast/normalize chain behind the
                # remaining apply matmuls
                for cc in (2, 3, 0, 1):
                    for p in pending:
                        apply_cc(*p, cc)
                pending = []

                # normalise: r[i] = GAMMA / (BETA sum_j e0[j,i]) (po3
                # partition 127 holds the ones-channel sum), u^ = u~ . r,
                # split hi/lo.  The rotation matmuls are deferred into the
                # next ib's loop so PE chews scores while this DVE/ACT
                # chain lands (last ib: emitted right here).
                r1 = rp.tile([1, NB], F32, name="r1", tag="r1")
                nc.vector.reciprocal(r1, po[2][0:1, :])
                rb = rp.tile([CK, NB], F32, name="rb", tag="rb")
                nc.gpsimd.partition_broadcast(rb, r1)
                uh = ubp.tile([CK, NCH, NB], F8, name="uh", tag="uh")
                ul = ubp.tile([CK, 2, NB], F8, name="ul", tag="ul")
                last_ib = ib == NIB - 1
                # (2,3,0,1): matches the flush order, so each u^ chunk is
                # normalized as soon as its accumulator lands
                for cc in (2, 3, 0, 1):
                    ft = ftp.tile([CK, NB], F32R, name="ft", tag="ft")
                    nc.vector.tensor_mul(ft, po[cc], rb)
                    if last_ib:
                        # ACT is idle at the tail; its lower latency
                        # shortens the final normalize->rotate chain
                        nc.scalar.activation(uh[:, cc, :], ft, AF.Copy)
                    else:
                        nc.gpsimd.tensor_copy(uh[:, cc, :], ft)
                    if cc < 2:
                        nc.vector.tensor_sub(ul[:, cc, :], ft, uh[:, cc, :])
                if ib < NIB - 1:
                    pending_rot = (uh, ul, ib)
                else:
                    emit_rotation(uh, ul, ib)

    nc.compile()
    return nc


_NC = None


def _get_nc():
    global _NC
    if _NC is None:
        _NC = build_bass()
    return _NC


def _split8(a):
    hi = np.asarray(a, NP8)
    lo = np.asarray(a - hi.astype(np.float32), NP8)
    return np.ascontiguousarray(np.stack([hi, lo]))


def _make_in_maps(inp, Wk, bk, Wq, bq, Wv, bv, Wo, bo):
    # host-side SVD rotations (f64) + folded weights; the scores bilinear
    # form is x_j^T M^T x_i, so rotate with the SVD of M^T
    M64 = (np.asarray(Wk, np.float64).T @ np.asarray(Wq, np.float64))
    U, sv, Vt = np.linalg.svd(M64.T)
    A64 = (np.asarray(Wo, np.float64) @ np.asarray(Wv, np.float64))
    Up, sp, Vpt = np.linalg.svd(A64)

    # h = (S V^T U) x~; kernel lhsT layout wants [c_in, k_out]
    Wh = (BETA * (np.diag(sv) @ Vt @ U)).astype(np.float32)
    wm2_ = _split8(np.ascontiguousarray(Wh.T))
    # b' = (S' V'^T U) x~ -> [c_in, k_out].  The ones channel must sit at
    # partition 0 of an apply psum chunk (engines can't start at partition
    # 127), so it lives at column 256: components 256..510 shift up one
    # column and the weakest component (sigma'_511) is dropped.
    Wb0 = (BETA * (U.T @ Vpt.T @ np.diag(sp))).astype(np.float32)
    Wb = np.zeros_like(Wb0)
    Wb[:, :256] = Wb0[:, :256]
    Wb[:, 257:] = Wb0[:, 256:C - 1]
    wv2_ = _split8(np.ascontiguousarray(Wb))
    # y = (BETA U')^T u^ / (BETA GAMMA); lhsT layout [k, c_out], rows
    # permuted to match (row 256 = ones channel = zero contribution)
    Rot0 = (BETA * Up.T).astype(np.float32)
    RotT = np.zeros_like(Rot0)
    RotT[:256] = Rot0[:256]
    RotT[257:] = Rot0[256:C - 1]
    rt2_ = _split8(np.ascontiguousarray(RotT))

    u_eff = (ALPHA * SCALE) * (U.T @ (np.asarray(Wq, np.float64).T
                                      @ np.asarray(bk, np.float64)))
    up2 = np.zeros((CK, NCH, 2), np.float32)
    up2[:, :, 0] = u_eff.astype(np.float32).reshape(NCH, CK).T
    uph_ = np.ascontiguousarray(up2.astype(NP8))

    bo_eff = (np.asarray(Wo, np.float32) @ np.asarray(bv, np.float32)
              + np.asarray(bo, np.float32))
    bop_ = np.ascontiguousarray(bo_eff.reshape(NCH, CK).T)

    x_all = np.asarray(inp, dtype=np.float32).reshape(B, C, HW)
    xsplit = [
        _split8((U.T @ x_all[b].astype(np.float64)).astype(np.float32))
        for b in range(B)
    ]

    in_maps = []
    for c in range(NCORES):
        b, h = divmod(c, NCORES // B)
        x2 = xsplit[b]
        in_maps.append({
            "xj2": x2,
            "xi2": np.ascontiguousarray(x2[:, :, h * I:(h + 1) * I]),
            "wm2": wm2_, "wv2": wv2_, "rt2": rt2_,
            "uph": uph_, "bop": bop_,
        })
    return in_maps


def run(trace=False, tmpdir=None, **inputs):
    nc = _get_nc()
    in_maps = _make_in_maps(**inputs)
    res = run_bass_kernel_spmd(
        nc, in_maps, core_ids=list(range(NCORES)), trace=trace, tmpdir=tmpdir
    )
    full = np.empty((B, C, HW), dtype=np.float32)
    for c in range(NCORES):
        b, h = divmod(c, NCORES // B)
        full[b][:, h * I:(h + 1) * I] = (
            res.results[c]["out"].astype(np.float32))
    return full.reshape(B, C, 64, 64), res


def kernel(**inputs):
    out, _ = run(trace=False, **inputs)
    return out


# revision 5
# speedup vs baseline: 1.0350x; 1.0006x over previous
"""AttnBlock2D (B=4, C=512, H=W=64) on 8 Trainium2 NeuronCores.

Data-parallel over batch x sequence-parallel over output tokens (core c
handles image c//2, output-token half c%2), with all heavy GEMMs in fp8e4m3
DoubleRow and *energy-ordered split precision* via host-side SVD rotations:

  scores:  s[i,j] = x_j^T (Wq^T Wk) x_i  (i-only terms cancel in softmax)
           Wq^T Wk ... M^T = U S V^T (SVD, host).  Send x~ = U^T x (iid
           N(0,1), same stats as x).  Then s = sum_k x~_jk h_ik with
           h = (S V^T U) x~_i, so component k carries energy S_k^2.  The
           top-256 components get the exact-ish 3-product hi/lo fp8
           expansion; the bottom-256 (1.5% of energy) get 1 product.
  apply:   u[c,i] = sum_j w_ij v'_cj, v' = (Wo Wv) x = U' S' V'^T x.
           b' = (S' V'^T U) x~ (sigma-folded, channel-ordered), apply
           contracts b' against e0 = fp8(exp(logits)) -- 2 products on the
           top-128 channels, 1 product elsewhere -- then y = U' (b-avg)
           back-rotation GEMM (top 3p / bottom 1p).  The e_lo correction is
           dropped entirely: numerator and denominator both use e0, which is
           exact softmax of logits perturbed by e's fp8 rounding (~1.2e-2).
  denom:   b' channel 256 is the constant BETA/GAMMA (memset; partition 0
           of the po2 chunk, where engines may read), so the apply GEMM
           also produces the softmax denominator for free and its plain
           DVE reciprocal is exactly the u^ normalize scale; the old
           ones-reduce matmuls are gone.

Per-core PE work drops from ~483k to ~271k cycles vs the 3-product
baseline; sim/HW rel-err 1.77e-2 (budget 2e-2; sim and HW agree to
+-3e-6 on this fixed-seed problem).

Layout/scheduling (evolved from the baseline): scores-transposed
formulation (softmax axis j on partitions, zero transposes), hi|lo packed
DMAs with half-chunk xj transfers need-ordered on the sync queue, b-proj/
pt riding ib0 with phase-A blocks at jcq 0/10/12/14 (matching their DMA
arrivals), p-state warm-up matmul, psum layout psA-ring(4)+po(4) = 8
banks with phase-A/back-rotation psums on the psA ring, per-ib
back-rotation matmuls deferred into the next ib (PE never waits on the
normalize chain), cc-major flush with the denominator chunk first, evac
work balanced ACT/DVE/Pool around the ScalarE exp stream.
"""

import numpy as np
import ml_dtypes

import concourse.bass as bass
import concourse.tile as tile
import concourse.mybir as mybir
from concourse import bacc
from concourse.bass_utils import run_bass_kernel_spmd

B = 4
C = 512            # C_IN == C_HID
HW = 64 * 64       # tokens per image
NCORES = 8
I = HW * B // NCORES   # 2048 output tokens per core

CK = 128           # partition chunk
NB = 512           # free-dim block
NCH = C // CK      # 4
NJB = HW // CK     # 32
NIB = I // NB      # 4
XCH = HW // 4      # xj DMA chunk (8 j-chunks)

F32 = mybir.dt.float32
F32R = mybir.dt.float32r
BF16 = mybir.dt.bfloat16
F8 = mybir.dt.float8e4
NP8 = ml_dtypes.float8_e4m3
AF = mybir.ActivationFunctionType
DR = mybir.MatmulPerfMode.DoubleRow
OP = mybir.AluOpType

SCALE = 1.0 / float(np.sqrt(float(C)))
BETA = 16.0        # weight pre-scale: keeps w_lo out of fp8 subnormals
ALPHA = 1024.0     # t-vector pre-scale
SHIFT = -2.0       # global logit shift (cancels in softmax; bounds e)
GAMMA = 16.0       # u^ pre-scale: keeps u^ out of fp8 subnormals

SKEW0 = 4          # ib0 apply skew (jcq)
SKEW = 5           # ib>0 apply skew: covers the u^ normalize DVE/ACT chain
ROT_AT = 4         # jcq at which the previous ib's rotation matmuls drop in


def build_bass():
    nc = bacc.Bacc(
        "TRN2", target_bir_lowering=False, debug=False, enable_asserts=False
    )

    # hi|lo packed inputs: one DMA per block
    xj2 = nc.dram_tensor("xj2", [2, C, HW], F8, kind="ExternalInput").ap()
    xi2 = nc.dram_tensor("xi2", [2, C, I], F8, kind="ExternalInput").ap()
    wm2 = nc.dram_tensor("wm2", [2, C, C], F8, kind="ExternalInput").ap()
    wv2 = nc.dram_tensor("wv2", [2, C, C], F8, kind="ExternalInput").ap()
    rt2 = nc.dram_tensor("rt2", [2, C, C], F8, kind="ExternalInput").ap()
    uph = nc.dram_tensor("uph", [CK, NCH, 2], F8, kind="ExternalInput").ap()
    bop = nc.dram_tensor("bop", [CK, NCH], F32, kind="ExternalInput").ap()
    out = nc.dram_tensor("out", [C, I], BF16, kind="ExternalOutput").ap()

    # DRAM views with the channel dim split for 128-partition DMA
    xj4 = xj2.rearrange("t (a p) n -> p t a n", p=CK)  # [128, 2, 4, 4096]
    xi4 = xi2.rearrange("t (a p) n -> p t a n", p=CK)  # [128, 2, 4, 2048]
    wm4 = wm2.rearrange("t (a p) n -> p t a n", p=CK)  # [128, 2, 4, 512]
    wv4 = wv2.rearrange("t (a p) n -> p t a n", p=CK)
    rt4 = rt2.rearrange("t (a p) n -> p t a n", p=CK)
    out3 = out.rearrange("(a p) n -> p a n", p=CK)     # [128, 4, 2048]

    with tile.TileContext(nc) as tc:
        with tc.tile_pool(name="persist", bufs=1) as persist, \
             tc.tile_pool(name="wp", bufs=1) as wp, \
             tc.tile_pool(name="xp", bufs=4) as xp, \
             tc.tile_pool(name="etp", bufs=8) as etp, \
             tc.tile_pool(name="ftp", bufs=4) as ftp, \
             tc.tile_pool(name="ubp", bufs=2) as ubp, \
             tc.tile_pool(name="rp", bufs=2) as rp, \
             tc.tile_pool(name="psA", bufs=4, space="PSUM") as psA, \
             tc.tile_pool(name="psO", bufs=1, space="PSUM") as psO, \
             tc.tile_pool(name="xj0p", bufs=1) as xj0p, \
             tc.tile_pool(name="xj1p", bufs=1) as xj1p, \
             tc.tile_pool(name="xj2p", bufs=1) as xj2p, \
             tc.tile_pool(name="xj3p", bufs=1) as xj3p:

            # ---- persistent SBUF state ----
            # each xj chunk gets its own single-tile pool so readers only
            # wait on the one chunk DMA that wrote their data (write
            # tracking is pool-granular)
            xjc = [p.tile([CK, 2, NCH, XCH], F8, name=f"xjc{c}")
                   for c, p in enumerate((xj0p, xj1p, xj2p, xj3p))]
            h_hi = persist.tile([CK, NCH, I], F8, name="h_hi")
            h_lo = persist.tile([CK, 2, I], F8, name="h_lo")   # top half only
            vT_hi = persist.tile([CK, NJB, C], F8, name="vT_hi")
            vT_lo = persist.tile([CK, NJB, CK], F8, name="vT_lo")
            tt = persist.tile([CK, NJB], F32, name="tt")
            bop_t = persist.tile([CK, NCH], F32, name="bop_t")
            # up_t lives in the weights pool: reading a tile waits on all
            # earlier writes to its pool, and persist receives the phase-A
            # h evacuations (which would stall the pt burst)
            up_t = wp.tile([CK, NCH, 2], F8, name="up_t")
            wm = wp.tile([CK, 2, NCH, C], F8, name="wm")
            wv = wp.tile([CK, 2, NCH, C], F8, name="wv")
            rt = wp.tile([CK, 2, NCH, C], F8, name="rt")

            # b' channel 256 == BETA/GAMMA: the apply GEMM's cc=2 chunk
            # then emits (BETA/GAMMA) sum_j e0[j,i] on po2 partition 0,
            # whose plain reciprocal is exactly the GAMMA/(BETA s) scale
            # the u^ normalize needs -- no separate rescale op
            nc.vector.memset(vT_hi[:, :, C // 2], BETA / GAMMA)
            # p-state warm-up: a tiny dummy matmul right after the memsets
            # starts the tensor engine's 3us ramp clock at ~0.2us, so the
            # first real matmuls (after the critical DMAs land) already run
            # at full rate
            warm = persist.tile([CK, 2, 32], F8, name="warm")
            nc.vector.memset(warm, BETA)
            dum = psA.tile([32, 32], F32, name="dum", tag="psA",
                           space="PSUM")
            nc.tensor.matmul(dum, lhsT=warm, rhs=warm,
                             start=True, stop=True, perf_mode=DR)

            # DMAs serialize globally in arrival order, so issue everything
            # need-ordered on the sync queue: phase-A inputs first, then the
            # xj chunks and wv interleaved in consumption order
            nc.sync.dma_start(out=wm[:, 0], in_=wm4[:, 0])

            # convenience pair views into the packed xj chunk tiles
            JPC = XCH // CK  # j-chunks per xj chunk tile

            def xjh_p(ccp, jc):
                return xjc[jc // JPC][:, 0, ccp:ccp + 2,
                                      (jc % JPC) * CK:(jc % JPC + 1) * CK]

            def xjl_p(ccp, jc):
                return xjc[jc // JPC][:, 1, ccp:ccp + 2,
                                      (jc % JPC) * CK:(jc % JPC + 1) * CK]

            # DMA stream (sync, need-ordered): wm0, xt0h, xt0l, wm1, xt1,
            # xjc0, xt2, xt3, wv, xjc1..3, rt.  Phase-A blocks themselves
            # run interleaved with ib-0's first four cycles (below).
            xts = []
            for ib in range(NIB):
                xt = xp.tile([CK, 2, NCH, NB], F8, name="xt", tag="xt")
                xts.append(xt)
            def xjc_dma(ch, half):
                lo = ch * XCH + half * (XCH // 2)
                nc.sync.dma_start(
                    out=xjc[ch][:, :, :, half * (XCH // 2):
                                (half + 1) * (XCH // 2)],
                    in_=xj4[:, :, :, lo:lo + XCH // 2])

            # need-ordered stream: phase-A(0) inputs, then xjc0 + wv so the
            # first scores/vt/apply pipeline saturates PE by ~10us; the
            # remaining xt blocks (phase-A 1-3, deferred to jcq 10-14) and
            # xj chunks follow in consumption order
            nc.sync.dma_start(out=xts[0][:, 0], in_=xi4[:, 0, :, 0:NB])
            nc.sync.dma_start(out=up_t, in_=uph)
            nc.sync.dma_start(out=xts[0][:, 1], in_=xi4[:, 1, :, 0:NB])
            nc.sync.dma_start(out=wm[:, 1], in_=wm4[:, 1])
            xjc_dma(0, 0)
            nc.sync.dma_start(out=wv[:, 0], in_=wv4[:, 0])
            xjc_dma(0, 1)
            nc.sync.dma_start(out=wv[:, 1], in_=wv4[:, 1])
            xjc_dma(1, 0)
            xjc_dma(1, 1)
            nc.gpsimd.dma_start(out=bop_t, in_=bop)
            nc.sync.dma_start(out=xts[1], in_=xi4[:, :, :, 1 * NB:2 * NB])
            nc.sync.dma_start(out=xts[2], in_=xi4[:, :, :, 2 * NB:3 * NB])
            nc.sync.dma_start(out=xts[3], in_=xi4[:, :, :, 3 * NB:4 * NB])
            for ch in range(2, 4):
                xjc_dma(ch, 0)
                xjc_dma(ch, 1)
            nc.sync.dma_start(out=rt, in_=rt4)

            def phase_a_block(ab):
                # h = (S V^T U) x~ for one 512-token block of own i.
                # Output rows (= h components) are energy-ordered: top co
                # chunks get the 3-product expansion, bottom 2 products.
                xt = xts[ab]
                for co in range(NCH):
                    # phase-A psums ride the psA ring so the blocks can run
                    # at any point of ib0; casts ACT, subs DVE
                    ph = psA.tile([CK, NB], F32, name=f"ph{co}",
                                  tag="psA", space="PSUM")
                    prods = (((0, 0), (0, 1), (1, 0)) if co < 2
                             else ((0, 0),))
                    first = True
                    for n, (wa, xa) in enumerate(prods):
                        for ccp in (0, 2):
                            nc.tensor.matmul(
                                ph,
                                lhsT=wm[:, wa, ccp:ccp + 2,
                                        co * CK:(co + 1) * CK],
                                rhs=xt[:, xa, ccp:ccp + 2, :],
                                start=first,
                                stop=(n == len(prods) - 1 and ccp == 2),
                                perf_mode=DR,
                            )
                            first = False
                    hh = h_hi[:, co, ab * NB:(ab + 1) * NB]
                    nc.scalar.activation(hh, ph, AF.Copy)
                    if co < 2:
                        hl = h_lo[:, co, ab * NB:(ab + 1) * NB]
                        nc.vector.tensor_sub(hl, ph, hh)

            def pt_burst(jc0, n=4):
                # t[j] = alpha.SCALE.(U^T Wq^T bk).x~_j  (hi-only product);
                # n j-chunks share one psum tile = one psA ring slot.  All
                # matmuls first, then ONE strided DVE op converts the
                # burst -- no PE<->DVE ping-pong on the critical path.
                pt = psA.tile([CK, n, 2], F32, name="pt", tag="psA",
                              space="PSUM")
                for k in range(n):
                    jc = jc0 + k
                    for ccp in (0, 2):
                        nc.tensor.matmul(
                            pt[:, k, :], lhsT=xjh_p(ccp, jc),
                            rhs=up_t[:, ccp:ccp + 2, :],
                            start=(ccp == 0), stop=(ccp == 2), perf_mode=DR,
                        )
                nc.vector.tensor_scalar(
                    tt[:, jc0:jc0 + n], pt[:, :, 0],
                    1.0 / ALPHA, SHIFT, OP.mult, OP.add,
                )

            def vt_gemm(jc):
                # b'[j, k] = (S' V'^T U) x~: top-256 output cols 3-product,
                # bottom-256 1-product; evac ACT hi (cols 0:511) / DVE lo
                # (cols 0:256 -- only the top needs a correction term)
                pv = psA.tile([CK, C], F32, name="pv", tag="psA",
                              space="PSUM")
                # one accumulation group per column range (they may not
                # interleave within a psum bank): top 3-product group first,
                # then the bottom 1-product group
                first = True
                for (xa, wa) in ((xjh_p, 0), (xjh_p, 1), (xjl_p, 0)):
                    for ccp in (0, 2):
                        nc.tensor.matmul(
                            pv[:, 0:C // 2], lhsT=xa(ccp, jc),
                            rhs=wv[:, wa, ccp:ccp + 2, 0:C // 2],
                            start=first,
                            stop=(xa is xjl_p and ccp == 2),
                            perf_mode=DR)
                        first = False
                nc.tensor.matmul(
                    pv[:, C // 2:C], lhsT=xjh_p(0, jc),
                    rhs=wv[:, 0, 0:2, C // 2:C],
                    start=True, stop=False, perf_mode=DR)
                nc.tensor.matmul(
                    pv[:, C // 2:C], lhsT=xjh_p(2, jc),
                    rhs=wv[:, 0, 2:4, C // 2:C],
                    start=False, stop=True, perf_mode=DR)
                # hi evac split ACT/DVE (GPSIMD cannot read PSUM): keeps
                # either engine under PE's ib0 rate.  Column 256 (the
                # memset ones channel) is skipped by both ranges.
                nc.scalar.activation(vT_hi[:, jc, 0:C // 2],
                                     pv[:, 0:C // 2], AF.Copy)
                nc.vector.tensor_scalar_add(vT_hi[:, jc, C // 2 + 1:C],
                                            pv[:, C // 2 + 1:C], 0.0)
                nc.vector.tensor_sub(vT_lo[:, jc, :], pv[:, 0:CK],
                                     vT_hi[:, jc, 0:CK])

            def emit_rotation(uh, ul, ib):
                # back-rotation y = (BETA U')^T u^ in 4 psum banks with
                # open accumulation groups: all top-component matmuls
                # (which only need uh/ul chunks 0,1) run first, so the
                # bottom matmuls wait on the late uh chunks 2,3 with the
                # PE already fed; evacs split DVE/ACT to shorten the tail
                # bottom-component products first: their u^ chunks (2,3)
                # finish earliest in the (2,3,0,1)-ordered flush, so these
                # matmuls feed PE while the top-set normalize still lands
                pys = []
                for co in range(NCH):
                    py = psA.tile([CK, NB], F32, name="py", tag="psA",
                                  space="PSUM")
                    pys.append(py)
                    nc.tensor.matmul(
                        py, lhsT=rt[:, 0, 2:4, co * CK:(co + 1) * CK],
                        rhs=uh[:, 2:4, :], start=True, stop=False,
                        perf_mode=DR)
                for co in range(NCH):
                    py = pys[co]
                    nc.tensor.matmul(
                        py, lhsT=rt[:, 0, 0:2, co * CK:(co + 1) * CK],
                        rhs=uh[:, 0:2, :], start=False, stop=False,
                        perf_mode=DR)
                    nc.tensor.matmul(
                        py, lhsT=rt[:, 0, 0:2, co * CK:(co + 1) * CK],
                        rhs=ul[:, 0:2, :], start=False, stop=False,
                        perf_mode=DR)
                    nc.tensor.matmul(
                        py, lhsT=rt[:, 1, 0:2, co * CK:(co + 1) * CK],
                        rhs=uh[:, 0:2, :], start=False, stop=True,
                        perf_mode=DR)
                    ftb = ftp.tile([CK, NB], BF16, name="ftb", tag="ftb")
                    if co % 2 == 0:
                        nc.vector.tensor_scalar(ftb, py,
                                                1.0 / (BETA * GAMMA),
                                                bop_t[:, co:co + 1],
                                                OP.mult, OP.add)
                    else:
                        nc.scalar.activation(ftb, py, AF.Identity,
                                             scale=1.0 / (BETA * GAMMA),
                                             bias=bop_t[:, co:co + 1])
                    nc.sync.dma_start(
                        out=out3[:, co, ib * NB:(ib + 1) * NB], in_=ftb)

            # ---- phase C: scores, exp, apply per 512-token i-block ----
            pending_rot = None
            for ib in range(NIB):
                po = [
                    psO.tile([CK, NB], F32, name=f"po{cc}", tag=f"po{cc}",
                             space="PSUM")
                    for cc in range(NCH)
                ]

                def apply_cc(jcq, et, cc):
                    # b'-lo correction only on the top-128 channels (cc=0):
                    # channels 128..255 carry ~13% of the energy, so their
                    # b-quantization noise is already small
                    jc0 = 2 * jcq
                    first = jcq == 0
                    last = jcq == NJB // 2 - 1
                    vh = vT_hi[:, jc0:jc0 + 2, cc * CK:(cc + 1) * CK]
                    nc.tensor.matmul(
                        po[cc], lhsT=vh, rhs=et[:, :, :],
                        start=first, stop=(last and cc != 0),
                        perf_mode=DR,
                    )
                    if cc == 0:
                        vl = vT_lo[:, jc0:jc0 + 2, cc * CK:(cc + 1) * CK]
                        nc.tensor.matmul(
                            po[cc], lhsT=vl, rhs=et[:, :, :],
                            start=False, stop=last, perf_mode=DR,
                        )

                def apply_jcq(jcq, et):
                    # u~[k, i] += b'[j, k] e0[j, i]: top chunks 2-product,
                    # bottom chunks 1-product (incl. the ones channel)
                    for cc in range(NCH):
                        apply_cc(jcq, et, cc)

                pending = []
                skew = SKEW0 if ib == 0 else SKEW
                for jcq in range(NJB // 2):
                    if ib == 0 and jcq == 0:
                        phase_a_block(0)
                    if ib == 0 and jcq in (10, 12, 14):
                        # phase-A blocks 1-3 ride ib-0's later cycles,
                        # well after their xt DMAs have landed
                        phase_a_block((jcq - 8) // 2)
                    if ib == 0 and jcq % 2 == 0:
                        pt_burst(2 * jcq, n=4)
                    if jcq == ROT_AT and pending_rot is not None:
                        # the previous ib's back-rotation matmuls drop in
                        # here, after a jcq of scores: by now the DVE/ACT
                        # u^ normalize+split chain has landed, so PE never
                        # waits on it
                        emit_rotation(*pending_rot)
                        pending_rot = None
                    et = etp.tile([CK, 2, NB], F8, name="et", tag="et")
                    for q in (0, 1):
                        jc = 2 * jcq + q
                        if ib == 0:
                            # b' before each scores half: interleaves the
                            # pv/ps psum ring and gives the phase-A evac
                            # chain time to land before scores reads h
                            vt_gemm(jc)
                        ps_ = psA.tile([CK, NB], F32, name="ps", tag="psA",
                                       space="PSUM")
                        # top components 3-product, bottom 1-product
                        nc.tensor.matmul(
                            ps_, lhsT=xjh_p(0, jc),
                            rhs=h_hi[:, 0:2, ib * NB:(ib + 1) * NB],
                            start=True, stop=False, perf_mode=DR)
                        nc.tensor.matmul(
                            ps_, lhsT=xjh_p(0, jc),
                            rhs=h_lo[:, 0:2, ib * NB:(ib + 1) * NB],
                            start=False, stop=False, perf_mode=DR)
                        # the x~-lo correction covers 3/4 of the i-block
                        # (F=384): cheaper, and the remaining quarter's
                        # noise is well inside the error budget
                        nc.tensor.matmul(
                            ps_[:, 0:384], lhsT=xjl_p(0, jc),
                            rhs=h_hi[:, 0:2, ib * NB:ib * NB + 384],
                            start=False, stop=False, perf_mode=DR)
                        nc.tensor.matmul(
                            ps_, lhsT=xjh_p(2, jc),
                            rhs=h_hi[:, 2:4, ib * NB:(ib + 1) * NB],
                            start=False, stop=True, perf_mode=DR)
                        # e0 = fp8(exp(logits)) straight from ScalarE
                        nc.scalar.activation(
                            et[:, q, :], ps_, AF.Exp,
                            scale=SCALE / BETA, bias=tt[:, jc:jc + 1])
                    pending.append((jcq, et))
                    # issue skew: PE runs scores(jcq+1..) while the ACT
                    # exp pipe finishes e0(jcq)
                    if len(pending) > skew:
                        apply_jcq(*pending.pop(0))
                # flush cc-major with the denominator chunk first: po2
                # (and then each po[cc]) completes early, hiding the
                # reciprocal/broadcast/normalize chain behind the
                # remaining apply matmuls
                for cc in (2, 3, 0, 1):
                    for p in pending:
                        apply_cc(*p, cc)
                pending = []

                # normalise: r[i] = GAMMA / (BETA sum_j e0[j,i]) (po3
                # partition 127 holds the ones-channel sum), u^ = u~ . r,
                # split hi/lo.  The rotation matmuls are deferred into the
                # next ib's loop so PE chews scores while this DVE/ACT
                # chain lands (last ib: emitted right here).
                r1 = rp.tile([1, NB], F32, name="r1", tag="r1")
                nc.vector.reciprocal(r1, po[2][0:1, :])
                rb = rp.tile([CK, NB], F32, name="rb", tag="rb")
                nc.gpsimd.partition_broadcast(rb, r1)
                uh = ubp.tile([CK, NCH, NB], F8, name="uh", tag="uh")
                ul = ubp.tile([CK, 2, NB], F8, name="ul", tag="ul")
                last_ib = ib == NIB - 1
                # (2,3,0,1): matches the flush order, so each u^ chunk is
                # normalized as soon as its accumulator lands
                for cc in (2, 3, 0, 1):
                    ft = ftp.tile([CK, NB], F32R, name="ft", tag="ft")
                    nc.vector.tensor_mul(ft, po[cc], rb)
                    if last_ib:
                        # ACT is idle at the tail; its lower latency
                        # shortens the final normalize->rotate chain
                        nc.scalar.activation(uh[:, cc, :], ft, AF.Copy)
                    else:
                        nc.gpsimd.tensor_copy(uh[:, cc, :], ft)
                    if cc < 2:
                        nc.vector.tensor_sub(ul[:, cc, :], ft, uh[:, cc, :])
                if ib < NIB - 1:
                    pending_rot = (uh, ul, ib)
                else:
                    emit_rotation(uh, ul, ib)

    nc.compile()
    return nc


_NC = None


def _get_nc():
    global _NC
    if _NC is None:
        _NC = build_bass()
    return _NC


def _split8(a):
    hi = np.asarray(a, NP8)
    lo = np.asarray(a - hi.astype(np.float32), NP8)
    return np.ascontiguousarray(np.stack([hi, lo]))


def _make_in_maps(inp, Wk, bk, Wq, bq, Wv, bv, Wo, bo):
    # host-side SVD rotations (f64) + folded weights; the scores bilinear
    # form is x_j^T M^T x_i, so rotate with the SVD of M^T
    M64 = (np.asarray(Wk, np.float64).T @ np.asarray(Wq, np.float64))
    U, sv, Vt = np.linalg.svd(M64.T)
    A64 = (np.asarray(Wo, np.float64) @ np.asarray(Wv, np.float64))
    Up, sp, Vpt = np.linalg.svd(A64)

    # h = (S V^T U) x~; kernel lhsT layout wants [c_in, k_out]
    Wh = (BETA * (np.diag(sv) @ Vt @ U)).astype(np.float32)
    wm2_ = _split8(np.ascontiguousarray(Wh.T))
    # b' = (S' V'^T U) x~ -> [c_in, k_out].  The ones channel must sit at
    # partition 0 of an apply psum chunk (engines can't start at partition
    # 127), so it lives at column 256: components 256..510 shift up one
    # column and the weakest component (sigma'_511) is dropped.
    Wb0 = (BETA * (U.T @ Vpt.T @ np.diag(sp))).astype(np.float32)
    Wb = np.zeros_like(Wb0)
    Wb[:, :256] = Wb0[:, :256]
    Wb[:, 257:] = Wb0[:, 256:C - 1]
    wv2_ = _split8(np.ascontiguousarray(Wb))
    # y = (BETA U')^T u^ / (BETA GAMMA); lhsT layout [k, c_out], rows
    # permuted to match (row 256 = ones channel = zero contribution)
    Rot0 = (BETA * Up.T).astype(np.float32)
    RotT = np.zeros_like(Rot0)
    RotT[:256] = Rot0[:256]
    RotT[257:] = Rot0[256:C - 1]
    rt2_ = _split8(np.ascontiguousarray(RotT))

    u_eff = (ALPHA * SCALE) * (U.T @ (np.asarray(Wq, np.float64).T
                                      @ np.asarray(bk, np.float64)))
    up2 = np.zeros((CK, NCH, 2), np.float32)
    up2[:, :, 0] = u_eff.astype(np.float32).reshape(NCH, CK).T
    uph_ = np.ascontiguousarray(up2.astype(NP8))

    bo_eff = (np.asarray(Wo, np.float32) @ np.asarray(bv, np.float32)
              + np.asarray(bo, np.float32))
    bop_ = np.ascontiguousarray(bo_eff.reshape(NCH, CK).T)

    x_all = np.asarray(inp, dtype=np.float32).reshape(B, C, HW)
    xsplit = [
        _split8((U.T @ x_all[b].astype(np.float64)).astype(np.float32))
        for b in range(B)
    ]

    in_maps = []
    for c in range(NCORES):
        b, h = divmod(c, NCORES // B)
        x2 = xsplit[b]
        in_maps.append({
            "xj2": x2,
            "xi2": np.ascontiguousarray(x2[:, :, h * I:(h + 1) * I]),
            "wm2": wm2_, "wv2": wv2_, "rt2": rt2_,
            "uph": uph_, "bop": bop_,
        })
    return in_maps


def run(trace=False, tmpdir=None, **inputs):
    nc = _get_nc()
    in_maps = _make_in_maps(**inputs)
    res = run_bass_kernel_spmd(
        nc, in_maps, core_ids=list(range(NCORES)), trace=trace, tmpdir=tmpdir
    )
    full = np.empty((B, C, HW), dtype=np.float32)
    for c in range(NCORES):
        b, h = divmod(c, NCORES // B)
        full[b][:, h * I:(h + 1) * I] = (
            res.results[c]["out"].astype(np.float32))
    return full.reshape(B, C, 64, 64), res


def kernel(**inputs):
    out, _ = run(trace=False, **inputs)
    return out


# revision 6
# speedup vs baseline: 1.0373x; 1.0021x over previous
"""AttnBlock2D (B=4, C=512, H=W=64) on 8 Trainium2 NeuronCores.

Data-parallel over batch x sequence-parallel over output tokens (core c
handles image c//2, output-token half c%2), with all heavy GEMMs in fp8e4m3
DoubleRow and *energy-ordered split precision* via host-side SVD rotations:

  scores:  s[i,j] = x_j^T (Wq^T Wk) x_i  (i-only terms cancel in softmax)
           Wq^T Wk ... M^T = U S V^T (SVD, host).  Send x~ = U^T x (iid
           N(0,1), same stats as x).  Then s = sum_k x~_jk h_ik with
           h = (S V^T U) x~_i, so component k carries energy S_k^2.  The
           top-256 components get the exact-ish 3-product hi/lo fp8
           expansion; the bottom-256 (1.5% of energy) get 1 product.
  apply:   u[c,i] = sum_j w_ij v'_cj, v' = (Wo Wv) x = U' S' V'^T x.
           b' = (S' V'^T U) x~ (sigma-folded, channel-ordered), apply
           contracts b' against e0 = fp8(exp(logits)) -- 2 products on the
           top-128 channels, 1 product elsewhere -- then y = U' (b-avg)
           back-rotation GEMM (top 3p / bottom 1p).  The e_lo correction is
           dropped entirely: numerator and denominator both use e0, which is
           exact softmax of logits perturbed by e's fp8 rounding (~1.2e-2).
  denom:   b' channel 256 is the constant BETA/GAMMA (memset; partition 0
           of the po2 chunk, where engines may read), so the apply GEMM
           also produces the softmax denominator for free and its plain
           DVE reciprocal is exactly the u^ normalize scale; the old
           ones-reduce matmuls are gone.

Per-core PE work drops from ~483k to ~271k cycles vs the 3-product
baseline; sim/HW rel-err 1.77e-2 (budget 2e-2; sim and HW agree to
+-3e-6 on this fixed-seed problem).

Layout/scheduling (evolved from the baseline): scores-transposed
formulation (softmax axis j on partitions, zero transposes), hi|lo packed
DMAs with half-chunk xj transfers need-ordered on the sync queue, b-proj/
pt riding ib0 with phase-A blocks at jcq 0/10/12/14 (matching their DMA
arrivals), p-state warm-up matmul, psum layout psA-ring(4)+po(4) = 8
banks with phase-A/back-rotation psums on the psA ring, per-ib
back-rotation matmuls deferred into the next ib (PE never waits on the
normalize chain), cc-major flush with the denominator chunk first, evac
work balanced ACT/DVE/Pool around the ScalarE exp stream.
"""

import numpy as np
import ml_dtypes

import concourse.bass as bass
import concourse.tile as tile
import concourse.mybir as mybir
from concourse import bacc
from concourse.bass_utils import run_bass_kernel_spmd

B = 4
C = 512            # C_IN == C_HID
HW = 64 * 64       # tokens per image
NCORES = 8
I = HW * B // NCORES   # 2048 output tokens per core

CK = 128           # partition chunk
NB = 512           # free-dim block
NCH = C // CK      # 4
NJB = HW // CK     # 32
NIB = I // NB      # 4
XCH = HW // 4      # xj DMA chunk (8 j-chunks)

F32 = mybir.dt.float32
F32R = mybir.dt.float32r
BF16 = mybir.dt.bfloat16
F8 = mybir.dt.float8e4
NP8 = ml_dtypes.float8_e4m3
AF = mybir.ActivationFunctionType
DR = mybir.MatmulPerfMode.DoubleRow
OP = mybir.AluOpType

SCALE = 1.0 / float(np.sqrt(float(C)))
BETA = 16.0        # weight pre-scale: keeps w_lo out of fp8 subnormals
ALPHA = 1024.0     # t-vector pre-scale
SHIFT = -2.0       # global logit shift (cancels in softmax; bounds e)
GAMMA = 16.0       # u^ pre-scale: keeps u^ out of fp8 subnormals

SKEW0 = 4          # ib0 apply skew (jcq)
SKEW = 5           # ib>0 apply skew: covers the u^ normalize DVE/ACT chain
ROT_AT = 4         # jcq at which the previous ib's rotation matmuls drop in


def build_bass():
    nc = bacc.Bacc(
        "TRN2", target_bir_lowering=False, debug=False, enable_asserts=False
    )

    # hi|lo packed inputs: one DMA per block
    xj2 = nc.dram_tensor("xj2", [2, C, HW], F8, kind="ExternalInput").ap()
    xi2 = nc.dram_tensor("xi2", [2, C, I], F8, kind="ExternalInput").ap()
    wm2 = nc.dram_tensor("wm2", [2, C, C], F8, kind="ExternalInput").ap()
    wv2 = nc.dram_tensor("wv2", [2, C, C], F8, kind="ExternalInput").ap()
    rt2 = nc.dram_tensor("rt2", [2, C, C], F8, kind="ExternalInput").ap()
    uph = nc.dram_tensor("uph", [CK, NCH, 2], F8, kind="ExternalInput").ap()
    bop = nc.dram_tensor("bop", [CK, NCH], F32, kind="ExternalInput").ap()
    out = nc.dram_tensor("out", [C, I], BF16, kind="ExternalOutput").ap()

    # DRAM views with the channel dim split for 128-partition DMA
    xj4 = xj2.rearrange("t (a p) n -> p t a n", p=CK)  # [128, 2, 4, 4096]
    xi4 = xi2.rearrange("t (a p) n -> p t a n", p=CK)  # [128, 2, 4, 2048]
    wm4 = wm2.rearrange("t (a p) n -> p t a n", p=CK)  # [128, 2, 4, 512]
    wv4 = wv2.rearrange("t (a p) n -> p t a n", p=CK)
    rt4 = rt2.rearrange("t (a p) n -> p t a n", p=CK)
    out3 = out.rearrange("(a p) n -> p a n", p=CK)     # [128, 4, 2048]

    with tile.TileContext(nc) as tc:
        with tc.tile_pool(name="persist", bufs=1) as persist, \
             tc.tile_pool(name="wp", bufs=1) as wp, \
             tc.tile_pool(name="xp", bufs=4) as xp, \
             tc.tile_pool(name="etp", bufs=8) as etp, \
             tc.tile_pool(name="ftp", bufs=4) as ftp, \
             tc.tile_pool(name="ubp", bufs=2) as ubp, \
             tc.tile_pool(name="rp", bufs=2) as rp, \
             tc.tile_pool(name="psA", bufs=4, space="PSUM") as psA, \
             tc.tile_pool(name="psO", bufs=1, space="PSUM") as psO, \
             tc.tile_pool(name="xj0p", bufs=1) as xj0p, \
             tc.tile_pool(name="xj1p", bufs=1) as xj1p, \
             tc.tile_pool(name="xj2p", bufs=1) as xj2p, \
             tc.tile_pool(name="xj3p", bufs=1) as xj3p:

            # ---- persistent SBUF state ----
            # each xj chunk gets its own single-tile pool so readers only
            # wait on the one chunk DMA that wrote their data (write
            # tracking is pool-granular)
            xjc = [p.tile([CK, 2, NCH, XCH], F8, name=f"xjc{c}")
                   for c, p in enumerate((xj0p, xj1p, xj2p, xj3p))]
            h_hi = persist.tile([CK, NCH, I], F8, name="h_hi")
            h_lo = persist.tile([CK, 2, I], F8, name="h_lo")   # top half only
            vT_hi = persist.tile([CK, NJB, C], F8, name="vT_hi")
            vT_lo = persist.tile([CK, NJB, CK], F8, name="vT_lo")
            tt = persist.tile([CK, NJB], F32, name="tt")
            bop_t = persist.tile([CK, NCH], F32, name="bop_t")
            # up_t lives in the weights pool: reading a tile waits on all
            # earlier writes to its pool, and persist receives the phase-A
            # h evacuations (which would stall the pt burst)
            up_t = wp.tile([CK, NCH, 2], F8, name="up_t")
            wm = wp.tile([CK, 2, NCH, C], F8, name="wm")
            wv = wp.tile([CK, 2, NCH, C], F8, name="wv")
            rt = wp.tile([CK, 2, NCH, C], F8, name="rt")

            # b' channel 256 == BETA/GAMMA: the apply GEMM's cc=2 chunk
            # then emits (BETA/GAMMA) sum_j e0[j,i] on po2 partition 0,
            # whose plain reciprocal is exactly the GAMMA/(BETA s) scale
            # the u^ normalize needs -- no separate rescale op
            nc.vector.memset(vT_hi[:, :, C // 2], BETA / GAMMA)
            # p-state warm-up: a tiny dummy matmul right after the memsets
            # starts the tensor engine's 3us ramp clock at ~0.2us, so the
            # first real matmuls (after the critical DMAs land) already run
            # at full rate
            warm = persist.tile([CK, 2, 32], F8, name="warm")
            nc.vector.memset(warm, BETA)
            dum = psA.tile([32, 32], F32, name="dum", tag="psA",
                           space="PSUM")
            nc.tensor.matmul(dum, lhsT=warm, rhs=warm,
                             start=True, stop=True, perf_mode=DR)

            # DMAs serialize globally in arrival order, so issue everything
            # need-ordered on the sync queue: phase-A inputs first, then the
            # xj chunks and wv interleaved in consumption order
            nc.sync.dma_start(out=wm[:, 0], in_=wm4[:, 0])

            # convenience pair views into the packed xj chunk tiles
            JPC = XCH // CK  # j-chunks per xj chunk tile

            def xjh_p(ccp, jc):
                return xjc[jc // JPC][:, 0, ccp:ccp + 2,
                                      (jc % JPC) * CK:(jc % JPC + 1) * CK]

            def xjl_p(ccp, jc):
                return xjc[jc // JPC][:, 1, ccp:ccp + 2,
                                      (jc % JPC) * CK:(jc % JPC + 1) * CK]

            # DMA stream (sync, need-ordered): wm0, xt0h, xt0l, wm1, xt1,
            # xjc0, xt2, xt3, wv, xjc1..3, rt.  Phase-A blocks themselves
            # run interleaved with ib-0's first four cycles (below).
            xts = []
            for ib in range(NIB):
                xt = xp.tile([CK, 2, NCH, NB], F8, name="xt", tag="xt")
                xts.append(xt)
            def xjc_dma(ch, half):
                lo = ch * XCH + half * (XCH // 2)
                nc.sync.dma_start(
                    out=xjc[ch][:, :, :, half * (XCH // 2):
                                (half + 1) * (XCH // 2)],
                    in_=xj4[:, :, :, lo:lo + XCH // 2])

            # need-ordered stream: phase-A(0) inputs, then xjc0 + wv so the
            # first scores/vt/apply pipeline saturates PE by ~10us; the
            # remaining xt blocks (phase-A 1-3, deferred to jcq 10-14) and
            # xj chunks follow in consumption order
            nc.sync.dma_start(out=xts[0][:, 0], in_=xi4[:, 0, :, 0:NB])
            nc.sync.dma_start(out=up_t, in_=uph)
            nc.sync.dma_start(out=xts[0][:, 1], in_=xi4[:, 1, :, 0:NB])
            nc.sync.dma_start(out=wm[:, 1], in_=wm4[:, 1])
            xjc_dma(0, 0)
            nc.sync.dma_start(out=wv[:, 0], in_=wv4[:, 0])
            xjc_dma(0, 1)
            nc.sync.dma_start(out=wv[:, 1], in_=wv4[:, 1])
            xjc_dma(1, 0)
            xjc_dma(1, 1)
            nc.gpsimd.dma_start(out=bop_t, in_=bop)
            nc.sync.dma_start(out=xts[1], in_=xi4[:, :, :, 1 * NB:2 * NB])
            nc.sync.dma_start(out=xts[2], in_=xi4[:, :, :, 2 * NB:3 * NB])
            nc.sync.dma_start(out=xts[3], in_=xi4[:, :, :, 3 * NB:4 * NB])
            for ch in range(2, 4):
                xjc_dma(ch, 0)
                xjc_dma(ch, 1)
            nc.sync.dma_start(out=rt, in_=rt4)

            def phase_a_block(ab):
                # h = (S V^T U) x~ for one 512-token block of own i.
                # Output rows (= h components) are energy-ordered: top co
                # chunks get the 3-product expansion, bottom 2 products.
                xt = xts[ab]
                for co in range(NCH):
                    # phase-A psums ride the psA ring so the blocks can run
                    # at any point of ib0; casts ACT, subs DVE
                    ph = psA.tile([CK, NB], F32, name=f"ph{co}",
                                  tag="psA", space="PSUM")
                    prods = (((0, 0), (0, 1), (1, 0)) if co < 2
                             else ((0, 0),))
                    first = True
                    for n, (wa, xa) in enumerate(prods):
                        for ccp in (0, 2):
                            nc.tensor.matmul(
                                ph,
                                lhsT=wm[:, wa, ccp:ccp + 2,
                                        co * CK:(co + 1) * CK],
                                rhs=xt[:, xa, ccp:ccp + 2, :],
                                start=first,
                                stop=(n == len(prods) - 1 and ccp == 2),
                                perf_mode=DR,
                            )
                            first = False
                    hh = h_hi[:, co, ab * NB:(ab + 1) * NB]
                    nc.scalar.activation(hh, ph, AF.Copy)
                    if co < 2:
                        hl = h_lo[:, co, ab * NB:(ab + 1) * NB]
                        nc.vector.tensor_sub(hl, ph, hh)

            def pt_burst(jc0, n=4):
                # t[j] = alpha.SCALE.(U^T Wq^T bk).x~_j  (hi-only product);
                # n j-chunks share one psum tile = one psA ring slot.  All
                # matmuls first, then ONE strided DVE op converts the
                # burst -- no PE<->DVE ping-pong on the critical path.
                pt = psA.tile([CK, n, 2], F32, name="pt", tag="psA",
                              space="PSUM")
                for k in range(n):
                    jc = jc0 + k
                    for ccp in (0, 2):
                        nc.tensor.matmul(
                            pt[:, k, :], lhsT=xjh_p(ccp, jc),
                            rhs=up_t[:, ccp:ccp + 2, :],
                            start=(ccp == 0), stop=(ccp == 2), perf_mode=DR,
                        )
                nc.vector.tensor_scalar(
                    tt[:, jc0:jc0 + n], pt[:, :, 0],
                    1.0 / ALPHA, SHIFT, OP.mult, OP.add,
                )

            def vt_gemm(jc):
                # b'[j, k] = (S' V'^T U) x~: top-256 output cols 3-product,
                # bottom-256 1-product; evac ACT hi (cols 0:511) / DVE lo
                # (cols 0:256 -- only the top needs a correction term)
                pv = psA.tile([CK, C], F32, name="pv", tag="psA",
                              space="PSUM")
                # one accumulation group per column range (they may not
                # interleave within a psum bank): top 3-product group first,
                # then the bottom 1-product group
                first = True
                for (xa, wa) in ((xjh_p, 0), (xjh_p, 1), (xjl_p, 0)):
                    for ccp in (0, 2):
                        nc.tensor.matmul(
                            pv[:, 0:C // 2], lhsT=xa(ccp, jc),
                            rhs=wv[:, wa, ccp:ccp + 2, 0:C // 2],
                            start=first,
                            stop=(xa is xjl_p and ccp == 2),
                            perf_mode=DR)
                        first = False
                nc.tensor.matmul(
                    pv[:, C // 2:C], lhsT=xjh_p(0, jc),
                    rhs=wv[:, 0, 0:2, C // 2:C],
                    start=True, stop=False, perf_mode=DR)
                nc.tensor.matmul(
                    pv[:, C // 2:C], lhsT=xjh_p(2, jc),
                    rhs=wv[:, 0, 2:4, C // 2:C],
                    start=False, stop=True, perf_mode=DR)
                # hi evac split ACT/DVE (GPSIMD cannot read PSUM): keeps
                # either engine under PE's ib0 rate.  Column 256 (the
                # memset ones channel) is skipped by both ranges.
                nc.scalar.activation(vT_hi[:, jc, 0:C // 2],
                                     pv[:, 0:C // 2], AF.Copy)
                nc.vector.tensor_scalar_add(vT_hi[:, jc, C // 2 + 1:C],
                                            pv[:, C // 2 + 1:C], 0.0)
                nc.vector.tensor_sub(vT_lo[:, jc, :], pv[:, 0:CK],
                                     vT_hi[:, jc, 0:CK])

            def emit_rotation(uh, ul, ib):
                # back-rotation y = (BETA U')^T u^ in 4 psum banks with
                # open accumulation groups: all top-component matmuls
                # (which only need uh/ul chunks 0,1) run first, so the
                # bottom matmuls wait on the late uh chunks 2,3 with the
                # PE already fed; evacs split DVE/ACT to shorten the tail
                # bottom-component products first: their u^ chunks (2,3)
                # finish earliest in the (2,3,0,1)-ordered flush, so these
                # matmuls feed PE while the top-set normalize still lands
                pys = []
                for co in range(NCH):
                    py = psA.tile([CK, NB], F32, name="py", tag="psA",
                                  space="PSUM")
                    pys.append(py)
                    nc.tensor.matmul(
                        py, lhsT=rt[:, 0, 2:4, co * CK:(co + 1) * CK],
                        rhs=uh[:, 2:4, :], start=True, stop=False,
                        perf_mode=DR)
                for co in range(NCH):
                    py = pys[co]
                    nc.tensor.matmul(
                        py, lhsT=rt[:, 0, 0:2, co * CK:(co + 1) * CK],
                        rhs=uh[:, 0:2, :], start=False, stop=False,
                        perf_mode=DR)
                    nc.tensor.matmul(
                        py, lhsT=rt[:, 0, 0:2, co * CK:(co + 1) * CK],
                        rhs=ul[:, 0:2, :], start=False, stop=False,
                        perf_mode=DR)
                    nc.tensor.matmul(
                        py, lhsT=rt[:, 1, 0:2, co * CK:(co + 1) * CK],
                        rhs=uh[:, 0:2, :], start=False, stop=True,
                        perf_mode=DR)
                    ftb = ftp.tile([CK, NB], BF16, name="ftb", tag="ftb")
                    if co % 2 == 0:
                        nc.vector.tensor_scalar(ftb, py,
                                                1.0 / (BETA * GAMMA),
                                                bop_t[:, co:co + 1],
                                                OP.mult, OP.add)
                    else:
                        nc.scalar.activation(ftb, py, AF.Identity,
                                             scale=1.0 / (BETA * GAMMA),
                                             bias=bop_t[:, co:co + 1])
                    nc.sync.dma_start(
                        out=out3[:, co, ib * NB:(ib + 1) * NB], in_=ftb)

            # ---- phase C: scores, exp, apply per 512-token i-block ----
            pending_rot = None
            for ib in range(NIB):
                po = [
                    psO.tile([CK, NB], F32, name=f"po{cc}", tag=f"po{cc}",
                             space="PSUM")
                    for cc in range(NCH)
                ]

                def apply_cc(jcq, et, cc):
                    # b'-lo correction only on the top-128 channels (cc=0):
                    # channels 128..255 carry ~13% of the energy, so their
                    # b-quantization noise is already small
                    jc0 = 2 * jcq
                    first = jcq == 0
                    last = jcq == NJB // 2 - 1
                    vh = vT_hi[:, jc0:jc0 + 2, cc * CK:(cc + 1) * CK]
                    nc.tensor.matmul(
                        po[cc], lhsT=vh, rhs=et[:, :, :],
                        start=first, stop=(last and cc != 0),
                        perf_mode=DR,
                    )
                    if cc == 0:
                        vl = vT_lo[:, jc0:jc0 + 2, cc * CK:(cc + 1) * CK]
                        nc.tensor.matmul(
                            po[cc], lhsT=vl, rhs=et[:, :, :],
                            start=False, stop=last, perf_mode=DR,
                        )

                def apply_jcq(jcq, et):
                    # u~[k, i] += b'[j, k] e0[j, i]: top chunks 2-product,
                    # bottom chunks 1-product (incl. the ones channel)
                    for cc in range(NCH):
                        apply_cc(jcq, et, cc)

                pending = []
                skew = SKEW0 if ib == 0 else SKEW
                for jcq in range(NJB // 2):
                    if ib == 0 and jcq == 0:
                        phase_a_block(0)
                    if ib == 0 and jcq in (10, 12, 14):
                        # phase-A blocks 1-3 ride ib-0's later cycles,
                        # well after their xt DMAs have landed
                        phase_a_block((jcq - 8) // 2)
                    if ib == 0 and jcq % 2 == 0:
                        pt_burst(2 * jcq, n=4)
                    if jcq == ROT_AT and pending_rot is not None:
                        # the previous ib's back-rotation matmuls drop in
                        # here, after a jcq of scores: by now the DVE/ACT
                        # u^ normalize+split chain has landed, so PE never
                        # waits on it
                        emit_rotation(*pending_rot)
                        pending_rot = None
                    et = etp.tile([CK, 2, NB], F8, name="et", tag="et")
                    for q in (0, 1):
                        jc = 2 * jcq + q
                        ps_ = psA.tile([CK, NB], F32, name="ps", tag="psA",
                                       space="PSUM")
                        # top components 3-product, bottom 1-product
                        nc.tensor.matmul(
                            ps_, lhsT=xjh_p(0, jc),
                            rhs=h_hi[:, 0:2, ib * NB:(ib + 1) * NB],
                            start=True, stop=False, perf_mode=DR)
                        nc.tensor.matmul(
                            ps_, lhsT=xjh_p(0, jc),
                            rhs=h_lo[:, 0:2, ib * NB:(ib + 1) * NB],
                            start=False, stop=False, perf_mode=DR)
                        # the x~-lo correction covers 3/4 of the i-block
                        # (F=384): cheaper, and the remaining quarter's
                        # noise is well inside the error budget
                        nc.tensor.matmul(
                            ps_[:, 0:384], lhsT=xjl_p(0, jc),
                            rhs=h_hi[:, 0:2, ib * NB:ib * NB + 384],
                            start=False, stop=False, perf_mode=DR)
                        nc.tensor.matmul(
                            ps_, lhsT=xjh_p(2, jc),
                            rhs=h_hi[:, 2:4, ib * NB:(ib + 1) * NB],
                            start=False, stop=True, perf_mode=DR)
                        # e0 = fp8(exp(logits)) straight from ScalarE
                        nc.scalar.activation(
                            et[:, q, :], ps_, AF.Exp,
                            scale=SCALE / BETA, bias=tt[:, jc:jc + 1])
                        if ib == 0:
                            # b' after each scores half: the exp (which
                            # gates the et->apply chain) queues on the
                            # ACT-bound ib0 stream ahead of the pv evac
                            vt_gemm(jc)
                    pending.append((jcq, et))
                    # issue skew: PE runs scores(jcq+1..) while the ACT
                    # exp pipe finishes e0(jcq)
                    if len(pending) > skew:
                        apply_jcq(*pending.pop(0))
                # flush cc-major with the denominator chunk first: po2
                # (and then each po[cc]) completes early, hiding the
                # reciprocal/broadcast/normalize chain behind the
                # remaining apply matmuls
                for cc in (2, 3, 0, 1):
                    for p in pending:
                        apply_cc(*p, cc)
                pending = []

                # normalise: r[i] = GAMMA / (BETA sum_j e0[j,i]) (po3
                # partition 127 holds the ones-channel sum), u^ = u~ . r,
                # split hi/lo.  The rotation matmuls are deferred into the
                # next ib's loop so PE chews scores while this DVE/ACT
                # chain lands (last ib: emitted right here).
                r1 = rp.tile([1, NB], F32, name="r1", tag="r1")
                nc.vector.reciprocal(r1, po[2][0:1, :])
                rb = rp.tile([CK, NB], F32, name="rb", tag="rb")
                nc.gpsimd.partition_broadcast(rb, r1)
                uh = ubp.tile([CK, NCH, NB], F8, name="uh", tag="uh")
                ul = ubp.tile([CK, 2, NB], F8, name="ul", tag="ul")
                last_ib = ib == NIB - 1
                # (2,3,0,1): matches the flush order, so each u^ chunk is
                # normalized as soon as its accumulator lands
                for cc in (2, 3, 0, 1):
                    ft = ftp.tile([CK, NB], F32R, name="ft", tag="ft")
                    nc.vector.tensor_mul(ft, po[cc], rb)
                    if last_ib:
                        # ACT is idle at the tail; its lower latency
                        # shortens the final normalize->rotate chain
                        nc.scalar.activation(uh[:, cc, :], ft, AF.Copy)
                    else:
                        nc.gpsimd.tensor_copy(uh[:, cc, :], ft)
                    if cc < 2:
                        nc.vector.tensor_sub(ul[:, cc, :], ft, uh[:, cc, :])
                if ib < NIB - 1:
                    pending_rot = (uh, ul, ib)
                else:
                    emit_rotation(uh, ul, ib)

    nc.compile()
    return nc


_NC = None


def _get_nc():
    global _NC
    if _NC is None:
        _NC = build_bass()
    return _NC


def _split8(a):
    hi = np.asarray(a, NP8)
    lo = np.asarray(a - hi.astype(np.float32), NP8)
    return np.ascontiguousarray(np.stack([hi, lo]))


def _make_in_maps(inp, Wk, bk, Wq, bq, Wv, bv, Wo, bo):
    # host-side SVD rotations (f64) + folded weights; the scores bilinear
    # form is x_j^T M^T x_i, so rotate with the SVD of M^T
    M64 = (np.asarray(Wk, np.float64).T @ np.asarray(Wq, np.float64))
    U, sv, Vt = np.linalg.svd(M64.T)
    A64 = (np.asarray(Wo, np.float64) @ np.asarray(Wv, np.float64))
    Up, sp, Vpt = np.linalg.svd(A64)

    # h = (S V^T U) x~; kernel lhsT layout wants [c_in, k_out]
    Wh = (BETA * (np.diag(sv) @ Vt @ U)).astype(np.float32)
    wm2_ = _split8(np.ascontiguousarray(Wh.T))
    # b' = (S' V'^T U) x~ -> [c_in, k_out].  The ones channel must sit at
    # partition 0 of an apply psum chunk (engines can't start at partition
    # 127), so it lives at column 256: components 256..510 shift up one
    # column and the weakest component (sigma'_511) is dropped.
    Wb0 = (BETA * (U.T @ Vpt.T @ np.diag(sp))).astype(np.float32)
    Wb = np.zeros_like(Wb0)
    Wb[:, :256] = Wb0[:, :256]
    Wb[:, 257:] = Wb0[:, 256:C - 1]
    wv2_ = _split8(np.ascontiguousarray(Wb))
    # y = (BETA U')^T u^ / (BETA GAMMA); lhsT layout [k, c_out], rows
    # permuted to match (row 256 = ones channel = zero contribution)
    Rot0 = (BETA * Up.T).astype(np.float32)
    RotT = np.zeros_like(Rot0)
    RotT[:256] = Rot0[:256]
    RotT[257:] = Rot0[256:C - 1]
    rt2_ = _split8(np.ascontiguousarray(RotT))

    u_eff = (ALPHA * SCALE) * (U.T @ (np.asarray(Wq, np.float64).T
                                      @ np.asarray(bk, np.float64)))
    up2 = np.zeros((CK, NCH, 2), np.float32)
    up2[:, :, 0] = u_eff.astype(np.float32).reshape(NCH, CK).T
    uph_ = np.ascontiguousarray(up2.astype(NP8))

    bo_eff = (np.asarray(Wo, np.float32) @ np.asarray(bv, np.float32)
              + np.asarray(bo, np.float32))
    bop_ = np.ascontiguousarray(bo_eff.reshape(NCH, CK).T)

    x_all = np.asarray(inp, dtype=np.float32).reshape(B, C, HW)
    xsplit = [
        _split8((U.T @ x_all[b].astype(np.float64)).astype(np.float32))
        for b in range(B)
    ]

    in_maps = []
    for c in range(NCORES):
        b, h = divmod(c, NCORES // B)
        x2 = xsplit[b]
        in_maps.append({
            "xj2": x2,
            "xi2": np.ascontiguousarray(x2[:, :, h * I:(h + 1) * I]),
            "wm2": wm2_, "wv2": wv2_, "rt2": rt2_,
            "uph": uph_, "bop": bop_,
        })
    return in_maps


def run(trace=False, tmpdir=None, **inputs):
    nc = _get_nc()
    in_maps = _make_in_maps(**inputs)
    res = run_bass_kernel_spmd(
        nc, in_maps, core_ids=list(range(NCORES)), trace=trace, tmpdir=tmpdir
    )
    full = np.empty((B, C, HW), dtype=np.float32)
    for c in range(NCORES):
        b, h = divmod(c, NCORES // B)
        full[b][:, h * I:(h + 1) * I] = (
            res.results[c]["out"].astype(np.float32))
    return full.reshape(B, C, 64, 64), res


def kernel(**inputs):
    out, _ = run(trace=False, **inputs)
    return out
